# revision 1
# baseline (speedup 1.0000x reference)
"""Trainium2 Bass kernel for nn_Discriminator: MLP + sort-based minibatch discrimination. Self-contained."""
import numpy as np

N = 2048
NROWS = 4
NCOLS = 512


def stages(n=None):
    if n is None:
        n = N
    out = []
    p = 1
    while p < n:
        k = p
        while k >= 1:
            lefts = []
            j = k % p
            while j <= n - 1 - k:
                for i in range(0, min(k, n - j - k)):
                    x = i + j
                    if (x // (2 * p)) == ((x + k) // (2 * p)):
                        lefts.append(x)
                j += 2 * k
            out.append((p, k, np.array(sorted(lefts), dtype=np.int64)))
            k //= 2
        p *= 2
    return out


def runs_of(xs):
    """Compress sorted ints into <=3-level pattern (start, L, s1, c1, s2, c2).
    Returns single tuple or None."""
    xs = np.asarray(xs)
    if len(xs) == 0:
        return None
    breaks = np.where(np.diff(xs) != 1)[0]
    starts_i = np.concatenate([[0], breaks + 1])
    ends_i = np.concatenate([breaks, [len(xs) - 1]])
    run_starts = xs[starts_i]
    run_lens = ends_i - starts_i + 1
    if not np.all(run_lens == run_lens[0]):
        return None
    L = int(run_lens[0])
    if len(run_starts) == 1:
        return (int(run_starts[0]), L, 0, 1, 0, 1)
    d = np.diff(run_starts)
    if np.all(d == d[0]):
        return (int(run_starts[0]), L, int(d[0]), len(run_starts), 0, 1)
    s1 = d[0]
    c1 = 1
    while c1 < len(d) and d[c1 - 1] == s1:
        c1 += 1
    group = c1
    if len(run_starts) % group != 0:
        return None
    rs = run_starts.reshape(-1, group)
    inner = np.diff(rs, axis=1)
    starts2 = rs[:, 0]
    d2 = np.diff(starts2)
    if inner.size and not np.all(inner == s1):
        return None
    if len(d2) and not np.all(d2 == d2[0]):
        return None
    return (int(run_starts[0]), L, int(s1), group,
            int(d2[0]) if len(d2) else 0, len(starts2))


def emit_ops():
    """Returns list of (p, k, [ops]); op = (r0, nrows, drow, colpat, colB0)
    colpat=(c0,L,s1,c1,s2,c2) for A; B cols = A cols + (colB0 - c0)."""
    all_stages = []
    for (p, k, lefts) in stages():
        ops = []
        rows = lefts // NCOLS
        cols = lefts % NCOLS
        drows = (lefts + k) // NCOLS - rows
        for dr in np.unique(drows):
            sel = drows == dr
            rset = np.unique(rows[sel])
            cset = np.unique(cols[sel])
            # must be Cartesian product
            assert sel.sum() == len(rset) * len(cset), (p, k, dr)
            for r in rset:
                cc = np.sort(cols[sel & (rows == r)])
                assert np.array_equal(cc, cset), (p, k, dr, r)
            colpat = runs_of(cset)
            assert colpat is not None, (p, k, dr, cset[:20])
            # split rset into contiguous runs
            rpat = runs_of(rset)
            assert rpat is not None, (p, k, dr, rset)
            (r0, Lr, sr1, cr1, sr2, cr2) = rpat
            assert sr2 == 0 and cr2 == 1, (p, k, dr, rpat)
            colB0 = int((cset[0] + k) % NCOLS)
            for g in range(cr1):
                rstart = r0 + g * sr1
                ops.append((int(rstart), int(Lr), int(dr), colpat, colB0))
        all_stages.append((p, int(k), ops))
    return all_stages


def _row_chunks(a_base, b_base, nr):
    """Split nr rows so every chunk's A and B partition ranges are legal:
    base 0 -> <=4 rows (128 parts), 32 -> 1, 64 -> 2, 96 -> 1 (32*r bases)."""
    allowed = {0: 4, 1: 1, 2: 2, 3: 1}
    out = []
    off = 0
    while off < nr:
        c = min(allowed[(a_base + off) % 4], allowed[(b_base + off) % 4], nr - off)
        out.append((off, c))
        off += c
    return out


def legalize(all_stages):
    """Split op-groups to satisfy partition-base rules."""
    out = []
    for (p, k, ops) in all_stages:
        nops = []
        for (r0, nr, dr, colpat, colB0) in ops:
            for (off, c) in _row_chunks(r0, r0 + dr, nr):
                nops.append((r0 + off, c, dr, colpat, colB0))
        out.append((p, k, nops))
    return out


def colpat_idx(colpat):
    (c0, L, s1, c1, s2, c2) = colpat
    return (c0 + np.arange(c2)[:, None, None] * s2
            + np.arange(c1)[None, :, None] * s1
            + np.arange(L)[None, None, :]).ravel()


def simulate(all_stages, arr):
    a = arr.copy()
    for (p, k, ops) in all_stages:
        for (r0, nr, dr, colpat, colB0) in ops:
            ia = colpat_idx(colpat)
            ib = ia + (colB0 - colpat[0])
            A = a[r0:r0 + nr, ia]
            Bm = a[r0 + dr:r0 + dr + nr, ib]
            mn = np.minimum(A, Bm)
            mx = np.maximum(A, Bm)
            a[r0:r0 + nr, ia] = mn
            a[r0 + dr:r0 + dr + nr, ib] = mx
    return a


def dve_cycles(all_stages):
    tot = 0.0
    n_ops = 0
    for (p, k, ops) in all_stages:
        for (r0, nr, dr, colpat, colB0) in ops:
            free = colpat[1] * colpat[3] * colpat[5]
            tot += (58 + free) * 2 + (58 + free / 2)
            n_ops += 3
    return tot, n_ops


def runs_multi(xs, max_groups=6):
    """Compress sorted ints into a list of <=3-level patterns.
    Groups runs by run-length first (complements of periodic patterns are
    unions of uniform-run periodic sets)."""
    xs = np.asarray(xs)
    if len(xs) == 0:
        return []
    r = runs_of(xs)
    if r is not None:
        return [r]
    breaks = np.where(np.diff(xs) != 1)[0]
    starts_i = np.concatenate([[0], breaks + 1])
    ends_i = np.concatenate([breaks, [len(xs) - 1]])
    run_starts = xs[starts_i]
    run_lens = ends_i - starts_i + 1
    out = []
    for L in np.unique(run_lens):
        sel = run_lens == L
        rs = run_starts[sel]
        # each group: runs of identical length -> starts should be periodic
        d = np.diff(rs)
        if len(d) == 0 or np.all(d == d[0]):
            out.append((int(rs[0]), int(L), int(d[0]) if len(d) else 0,
                        len(rs), 0, 1))
        else:
            # fall back: one op per run
            for s in rs:
                out.append((int(s), int(L), 0, 1, 0, 1))
    return out


def emit_pingpong():
    """Stages with compare ops + complement copy ops for ping-pong buffers.
    Returns list of (p, k, cmp_ops, cp_ops):
      cmp op: (r0, nr, dr, colpat, colB0)
      cp op:  (r0, nr, colpat)
    """
    out = []
    for (p, k, ops) in legalize(emit_ops()):
        touched = np.zeros((NROWS, NCOLS), dtype=bool)
        for (r0, nr, dr, colpat, colB0) in ops:
            ia = colpat_idx(colpat)
            ib = ia + (colB0 - colpat[0])
            for rr in range(r0, r0 + nr):
                touched[rr, ia] = True
                touched[rr + dr, ib] = True
        cp_ops = []
        # group contiguous rows with identical complement masks
        r = 0
        while r < NROWS:
            mask = ~touched[r]
            r2 = r + 1
            while r2 < NROWS and np.array_equal(~touched[r2], mask):
                r2 += 1
            cols = np.where(mask)[0]
            if len(cols):
                for pat in runs_multi(cols):
                    # legal row chunks for 1-input ops (A base only)
                    off = 0
                    nr_ = r2 - r
                    allowed = {0: 4, 1: 1, 2: 2, 3: 1}
                    while off < nr_:
                        c = min(allowed[(r + off) % 4], nr_ - off)
                        cp_ops.append((r + off, c, pat))
                        off += c
            r = r2
        out.append((p, k, ops, cp_ops))
    return out


def simulate_pp(stages_pp, arr):
    """Ping-pong simulation: validates full coverage each stage."""
    cur = arr.copy()
    for (p, k, cmp_ops, cp_ops) in stages_pp:
        nxt = np.full_like(cur, np.nan)
        for (r0, nr, dr, colpat, colB0) in cmp_ops:
            ia = colpat_idx(colpat)
            ib = ia + (colB0 - colpat[0])
            A = cur[r0:r0 + nr, ia]
            Bm = cur[r0 + dr:r0 + dr + nr, ib]
            nxt[r0:r0 + nr, ia] = np.minimum(A, Bm)
            nxt[r0 + dr:r0 + dr + nr, ib] = np.maximum(A, Bm)
        for (r0, nr, pat) in cp_ops:
            ic = colpat_idx(pat)
            nxt[r0:r0 + nr, ic] = cur[r0:r0 + nr, ic]
        assert not np.isnan(nxt).any(), (p, k, "coverage hole")
        cur = nxt
    return cur


if __name__ == "__main__" or True:
    pass


def _split_colpat(colpat, max_free=288):
    """Split a colpat into pieces each with free size <= max_free.
    Returns list of (delta_offset, colpat)."""
    (c0, L, s1, c1, s2, c2) = colpat
    free = L * c1 * c2
    if free <= max_free:
        return [(0, colpat)]
    if c2 > 1:
        h = c2 // 2
        a = (c0, L, s1, c1, s2, h)
        b = (c0 + h * s2, L, s1, c1, s2, c2 - h)
        return [(d, p) for d0, pp_ in [(0, a), (h * s2, b)]
                for d, p in [(d0 + dd, p2) for dd, p2 in _split_colpat(
                    (pp_[0], pp_[1], pp_[2], pp_[3], pp_[4], pp_[5]), max_free)]]
    if c1 > 1:
        h = c1 // 2
        a = (c0, L, s1, h, 0, 1)
        b = (c0 + h * s1, L, s1, c1 - h, 0, 1)
        out = []
        for base, pat in [(0, a), (h * s1, b)]:
            out.extend(_split_colpat(pat, max_free))
        return out
    h = L // 2
    a = (c0, h, 0, 1, 0, 1)
    b = (c0 + h, L - h, 0, 1, 0, 1)
    return _split_colpat(a, max_free) + _split_colpat(b, max_free)


def drain_split(stages_pp, max_free=288):
    """Split big cmp/copy ops so DVE drain overhead stays bounded."""
    out = []
    for (p, k, cmp_ops, cp_ops) in stages_pp:
        nc_ops = []
        for (r0, nr, dr, colpat, colB0) in cmp_ops:
            for (_, pat) in _split_colpat(colpat, max_free):
                nb0 = colB0 + (pat[0] - colpat[0])
                nc_ops.append((r0, nr, dr, pat, nb0))
        ncp_ops = []
        for (r0, nr, pat) in cp_ops:
            for (_, p2) in _split_colpat(pat, max_free):
                ncp_ops.append((r0, nr, p2))
        out.append((p, k, nc_ops, ncp_ops))
    return out


def gen_pingpong(n, nrows, ncols, p_min=1, max_free=288):
    """Parametric ping-pong network for n = nrows*ncols fold, phases p >= p_min."""
    global N, NROWS, NCOLS
    oldN, oldR, oldC = N, NROWS, NCOLS
    N, NROWS, NCOLS = n, nrows, ncols
    try:
        full = emit_pingpong()
        filt = [(p, k, c, cp) for (p, k, c, cp) in full if p >= p_min]
        return drain_split(filt, max_free)
    finally:
        N, NROWS, NCOLS = oldN, oldR, oldC



import bass_rust
import concourse.bacc as bacc
import concourse.mybir as mybir
from concourse import tile
from concourse.bass_utils import run_bass_kernel_spmd
from concourse.masks import make_identity


B, D, H1, H2, F = 2048, 3072, 512, 256, 100
NCORES = 8
BS = B // NCORES            # 256 rows per core
LEAK = 0.2
P = 128
FL = 13                     # features per core (8*13 = 104 >= 100)
FPAD = NCORES * FL          # 104
NR, NC = NROWS, NCOLS   # 4, 512
RC = 2.0 ** 23              # rounding constant
QLEV = 8190.0               # quantization levels (margin below 2^13)
MRANGE = 16.0               # fixed m-quantization range [-16, 16)
QSCALE = QLEV / (2 * MRANGE)
DQ = (2 * MRANGE) / QLEV
FSCALE = QLEV / 2048.0
FDEC = 2048.0 / QLEV

f32 = mybir.dt.float32
AF = mybir.ActivationFunctionType
ALU = mybir.AluOpType

SORT_OPS = legalize(emit_ops())


def sap(t_ap, pitch, pstart, pcount, coff, colpat):
    """Build a strided AP view: partitions [pstart, pstart+pcount), free
    pattern colpat=(c0,L,s1,c1,s2,c2) shifted to coff."""
    (c0, L, s1, c1, s2, c2) = colpat
    dims = [(pitch, pcount)]
    if c2 > 1:
        dims.append((s2, c2))
    if c1 > 1:
        dims.append((s1, c1))
    dims.append((1, L))
    a = t_ap.copy()
    a.ap = bass_rust.VecI64Pair(dims)
    a.offset = pstart * pitch + coff
    return a


SRC_OPS = gen_pingpong(256, 1, 256)
MRG_OPS = gen_pingpong(2048, 4, 512, p_min=256)
FULL_OPS = gen_pingpong(2048, 4, 512)


def emit_sort(nc, ops_table, rowpart, buf, tmp, pmirror, pitch, ppitch,
              mirror_copy, cp_engines, mir_pool=None):
    """Ping-pong odd-even mergesort between `buf` and `tmp` ([128, NC] tiles).
    Each stage: DVE min/max write the other buffer; untouched cells are
    copied across by cp_engines (ACT/GPSIMD); cross-row compares read the
    B operand through a PSUM mirror (mirror_copy must reach PSUM).
    len(SORT_PP) is even, so the result lands back in `buf`."""
    bufs = [buf, tmp]
    ci = 0
    for si, (p, k, cmp_ops, cp_ops) in enumerate(ops_table):
        cur = bufs[si % 2]
        nxt = bufs[(si + 1) % 2]
        for (r0, nr, dr, colpat, colB0) in cmp_ops:
            pa, pb = rowpart * r0, rowpart * (r0 + dr)
            npart = rowpart * nr
            a_in = sap(cur, pitch, pa, npart, colpat[0], colpat)
            a_out = sap(nxt, pitch, pa, npart, colpat[0], colpat)
            b_out = sap(nxt, pitch, pb, npart, colB0, colpat)
            if dr == 0:
                b_in = sap(cur, pitch, pb, npart, colB0, colpat)
            else:
                b_cur = sap(cur, pitch, pb, npart, colB0, colpat)
                if mir_pool is not None:
                    mt = mir_pool.tile([128, 512], mybir.dt.float32,
                                       tag="mir", bufs=6, name="mirt")
                    b_in = sap(mt[:], mt[:].ap[0][0], pa, npart,
                               colpat[0], colpat)
                else:
                    b_in = sap(pmirror, ppitch, pa, npart, colpat[0], colpat)
                mirror_copy(b_in, b_cur)
            nc.vector.tensor_tensor(a_out, a_in, b_in, ALU.min)
            nc.vector.tensor_tensor(b_out, a_in, b_in, ALU.max)
        for (r0, nr, pat) in cp_ops:
            pa = rowpart * r0
            npart = rowpart * nr
            c_in = sap(cur, pitch, pa, npart, pat[0], pat)
            c_out = sap(nxt, pitch, pa, npart, pat[0], pat)
            cp_engines[ci % len(cp_engines)](c_out, c_in)
            ci += 1


def build_program(dbg=False, repeat=1, upto=99):
    nc = bacc.Bacc(
        "TRN2", target_bir_lowering=False, debug=False, num_devices=NCORES)

    xT = nc.dram_tensor("xT", [D, BS], f32, kind="ExternalInput").ap()
    W1 = nc.dram_tensor("W1", [D, H1], f32, kind="ExternalInput").ap()
    b1c = nc.dram_tensor("b1c", [H1, 1], f32, kind="ExternalInput").ap()
    W2 = nc.dram_tensor("W2", [H1, H2], f32, kind="ExternalInput").ap()
    b2c = nc.dram_tensor("b2c", [H2, 1], f32, kind="ExternalInput").ap()
    Tm = nc.dram_tensor("Tm", [H2, F], f32, kind="ExternalInput").ap()
    Wfh = nc.dram_tensor("Wfh", [H2, 1], f32, kind="ExternalInput").ap()
    Wffl = nc.dram_tensor("Wffl", [FL, 1], f32, kind="ExternalInput").ap()
    bfc = nc.dram_tensor("bfc", [1, 1], f32, kind="ExternalInput").ap()
    ones1 = nc.dram_tensor("ones1", [1, P], f32, kind="ExternalInput").ap()
    ones8 = nc.dram_tensor("ones8", [NCORES, 1], f32, kind="ExternalInput").ap()
    iotas = nc.dram_tensor("iotas", [P, BS], f32, kind="ExternalInput").ap()
    out = nc.dram_tensor("out", [1, B], f32, kind="ExternalOutput").ap()
    outh = nc.dram_tensor("outh", [1, BS], f32, kind="ExternalOutput").ap()

    dbg_aps = {}
    if dbg:
        for nm, shp in [("d_msort", [P, NC]), ("d_key", [P, NC]),
                        ("d_sorted", [P, NC]), ("d_u", [P, NC]),
                        ("d_s1u", [P, NC]), ("d_s2v", [P, NC]),
                        ("d_feats", [P, NC]), ("d_key2s", [P, NC]),
                        ("d_fdec", [P, NC]), ("d_scal", [P, 8]),
                        ("d_contrib", [1, B]), ("d_f13", [FL, B])]:
            dbg_aps[nm] = nc.dram_tensor(nm, shp, f32, kind="ExternalOutput").ap()

    KD, K1, K2 = D // P, H1 // P, H2 // P

    with tile.TileContext(nc) as tc:
      for _rep in range(repeat):
        with (
            tc.tile_pool(name="persist", bufs=1) as pers,
            tc.tile_pool(name="dram", bufs=1, space="DRAM") as dpool,
        ):
            # ======== persistent small tiles ========
            ident_sb = pers.tile([P, P], f32)
            make_identity(nc, ident_sb[:])
            ones_sb = pers.tile([1, P], f32)
            nc.sync.dma_start(ones_sb[:], ones1)
            ones8_sb = pers.tile([NCORES, 1], f32)
            nc.sync.dma_start(ones8_sb[:], ones8)
            bf_sb = pers.tile([1, 1], f32)
            nc.sync.dma_start(bf_sb[:], bfc)
            Wffl_sb = pers.tile([P, 1], f32)
            for q in range(4):
                nc.sync.dma_start(Wffl_sb[32 * q:32 * q + FL, :], Wffl)
            iota_sb = pers.tile([P, BS], f32)
            nc.sync.dma_start(iota_sb[:], iotas)
            hWf_sb = pers.tile([1, BS], f32)
            mT_loc = pers.tile([F, BS], f32)

            # ======== phase 1: MLP ========
            with (
                tc.tile_pool(name="mlp", bufs=1) as mp,
                tc.tile_pool(name="psum_mm", bufs=2, space="PSUM") as pmm,
            ):
                f32r = mybir.dt.float32r
                xT_sb = [mp.tile([P, BS], f32r, name=f"xT{k}") for k in range(KD)]
                W1_sb = [mp.tile([P, H1], f32r, name=f"W1s{k}") for k in range(KD)]
                for k in range(KD):
                    stx = mp.tile([P, BS], f32, tag="stgx", bufs=3,
                                  name=f"stx{k}")
                    nc.sync.dma_start(stx[:], xT[k * P:(k + 1) * P, :])
                    nc.scalar.copy(xT_sb[k][:], stx[:])
                    stw = mp.tile([P, H1], f32, tag="stgw", bufs=3,
                                  name=f"stw{k}")
                    nc.sync.dma_start(stw[:], W1[k * P:(k + 1) * P, :])
                    nc.vector.tensor_copy(W1_sb[k][:], stw[:])
                W2_sb = [mp.tile([P, H2], f32, name=f"W2s{k}") for k in range(K1)]
                for k in range(K1):
                    nc.sync.dma_start(W2_sb[k][:], W2[k * P:(k + 1) * P, :])
                T_sb = [mp.tile([P, F], f32, name=f"Ts{k}") for k in range(K2)]
                for k in range(K2):
                    nc.sync.dma_start(T_sb[k][:], Tm[k * P:(k + 1) * P, :])
                Wfh_sb = mp.tile([P, K2], f32)
                nc.sync.dma_start(
                    Wfh_sb[:], Wfh.rearrange("(k p) one -> p (k one)", p=P))
                b1_sb = mp.tile([P, K1], f32)
                nc.sync.dma_start(
                    b1_sb[:], b1c.rearrange("(k p) one -> p (k one)", p=P))
                b2_sb = mp.tile([P, K2], f32)
                nc.sync.dma_start(
                    b2_sb[:], b2c.rearrange("(k p) one -> p (k one)", p=P))

                h1T = [mp.tile([P, BS], f32, name=f"h1T{m}") for m in range(K1)]
                for mb in range(K1):
                    pt = pmm.tile([P, BS], f32, tag="mm")
                    for k in range(KD):
                        nc.tensor.matmul(
                            pt[:], W1_sb[k][:, mb * P:(mb + 1) * P], xT_sb[k][:],
                            start=(k == 0), stop=(k == KD - 1))
                    s1 = mp.tile([P, BS], f32, tag="stmp", bufs=2, name=f"s1_{mb}")
                    nc.scalar.activation(
                        s1[:], pt[:], AF.Identity, bias=b1_sb[:, mb:mb + 1])
                    nc.vector.scalar_tensor_tensor(
                        h1T[mb][:], s1[:], LEAK, s1[:], op0=ALU.mult, op1=ALU.max)

                h2T = [mp.tile([P, BS], f32, name=f"h2T{m}") for m in range(K2)]
                for mb in range(K2):
                    pt = pmm.tile([P, BS], f32, tag="mm")
                    for k in range(K1):
                        nc.tensor.matmul(
                            pt[:], W2_sb[k][:, mb * P:(mb + 1) * P], h1T[k][:],
                            start=(k == 0), stop=(k == K1 - 1))
                    s2 = mp.tile([P, BS], f32, tag="stmp", bufs=2, name=f"s2_{mb}")
                    nc.scalar.activation(
                        s2[:], pt[:], AF.Identity, bias=b2_sb[:, mb:mb + 1])
                    nc.vector.scalar_tensor_tensor(
                        h2T[mb][:], s2[:], LEAK, s2[:], op0=ALU.mult, op1=ALU.max)

                pt_m = pmm.tile([F, BS], f32, tag="mm")
                for k in range(K2):
                    nc.tensor.matmul(
                        pt_m[:], T_sb[k][:], h2T[k][:],
                        start=(k == 0), stop=(k == K2 - 1))
                nc.scalar.copy(mT_loc[:], pt_m[:])

                ph = pmm.tile([1, BS], f32, tag="hw")
                for k in range(K2):
                    nc.tensor.matmul(
                        ph[:], Wfh_sb[:, k:k + 1], h2T[k][:],
                        start=(k == 0), stop=(k == K2 - 1))
                nc.vector.tensor_copy(hWf_sb[:], ph[:])

            if upto <= 1:
                nc.sync.dma_start(out[:, 0:BS], mT_loc[0:1, :])
                continue
            # ======== phase 2: AllToAll #1 ========
            skey = pers.tile([P, BS], f32)
            sktmp = pers.tile([P, BS], f32)
            nc.vector.memset(skey[:], 0.0)
            nc.vector.tensor_scalar(
                skey[:F, :], mT_loc[:], scalar1=MRANGE, scalar2=QSCALE,
                op0=ALU.add, op1=ALU.mult)
            nc.vector.tensor_scalar(
                skey[:F, :], skey[:F, :], scalar1=RC, scalar2=RC,
                op0=ALU.add, op1=ALU.subtract)
            nc.vector.tensor_scalar(
                skey[:F, :], skey[:F, :], scalar1=8191.0, scalar2=0.0,
                op0=ALU.min, op1=ALU.max)
            nc.vector.tensor_tensor(skey[:F, :], skey[:F, :], iota_sb[:F, :],
                                    ALU.add)
            spitch = skey[:].ap[0][0]
            emit_sort(nc, SRC_OPS, P, skey[:], sktmp[:], None, spitch, 0,
                      None,
                      [lambda o, i: nc.scalar.copy(o, i),
                       lambda o, i: nc.gpsimd.tensor_copy(o, i)])
            a2a_in = dpool.tile([FPAD, BS], f32)
            a2a_out = dpool.tile([FPAD, BS], f32)
            nc.sync.dma_start(a2a_in[:F, :], skey[:F, :])
            # rows 100:104 = copies of features 0:4 (benign padding)
            nc.sync.dma_start(a2a_in[F:FPAD, :], skey[:FPAD - F, :])
            nc.gpsimd.collective_compute(
                "AllToAll", ALU.bypass,
                replica_groups=[list(range(NCORES))],
                ins=[a2a_in.opt()], outs=[a2a_out.opt()])

            key = pers.tile([P, NC], f32)
            nc.vector.memset(key[:], 0.0)
            for e in range(NCORES):
                r, half = e // 2, e % 2
                nc.sync.dma_start(
                    key[32 * r:32 * r + FL, half * BS:(half + 1) * BS],
                    a2a_out[e * FL:(e + 1) * FL, :])

            # ======== phase 3 ========
            pitch = key[:].ap[0][0]
            with (
                tc.tile_pool(name="sortp", bufs=1) as sp,
                tc.tile_pool(name="psum2", bufs=1, space="PSUM") as pp2,
            ):
                # ======== phase 4: merge (30 stages) ========
                tmp = sp.tile([P, NC], f32)
                pmir = pp2.tile([P, NC], f32, tag="mir", bufs=6, name="pmir")
                ppitch = pmir[:].ap[0][0]
                emit_sort(nc, MRG_OPS, 32, key[:], tmp[:], pmir[:], pitch,
                          ppitch,
                          mirror_copy=lambda o, i: nc.scalar.copy(o, i),
                          cp_engines=[lambda o, i: nc.gpsimd.tensor_copy(o, i),
                                      lambda o, i: nc.scalar.copy(o, i)],
                          mir_pool=pp2)
                if dbg:
                    nc.sync.dma_start(dbg_aps["d_sorted"][:], key[:])

                if upto <= 4:
                    nc.sync.dma_start(out[:, 0:BS], key[0:1, 0:BS])
                    continue
                # ======== phase 5: feats in sorted order ========
                # split key = g + j/2048 via integer masking of key*2048
                ki = sp.tile([P, NC], mybir.dt.int32)
                kq = sp.tile([P, NC], f32)
                nc.vector.tensor_scalar_mul(kq[:], key[:], 2048.0)
                nc.vector.tensor_copy(ki[:], kq[:])
                ji = sp.tile([P, NC], mybir.dt.int32)
                nc.vector.tensor_scalar(
                    ji[:], ki[:], scalar1=2047, scalar2=None, op0=ALU.bitwise_and)
                jf = sp.tile([P, NC], f32)
                nc.vector.tensor_copy(jf[:], ji[:])
                gi = sp.tile([P, NC], mybir.dt.int32)
                nc.vector.tensor_scalar(
                    gi[:], ki[:], scalar1=-2048, scalar2=None, op0=ALU.bitwise_and)
                g2k = sp.tile([P, NC], f32)   # g * 2048
                nc.vector.tensor_copy(g2k[:], gi[:])
                bneg = sp.tile([P, 1], f32)
                nc.vector.memset(bneg[:], -MRANGE)
                bpos = sp.tile([P, 1], f32)
                nc.vector.memset(bpos[:], MRANGE)
                u = sp.tile([P, NC], f32)
                nc.scalar.activation(
                    u[:], g2k[:], AF.Exp, bias=bneg[:], scale=DQ / 2048.0)
                v = sp.tile([P, NC], f32)
                nc.scalar.activation(
                    v[:], g2k[:], AF.Exp, bias=bpos[:], scale=-DQ / 2048.0)

                su = sp.tile([P, NC], f32)
                nc.vector.tensor_tensor_scan(
                    su[:], u[:], u[:], initial=0.0, op0=ALU.add, op1=ALU.bypass)
                sv = sp.tile([P, NC], f32)
                nc.vector.tensor_tensor_scan(
                    sv[:, NC - 1::-1], v[:, NC - 1::-1], v[:, NC - 1::-1],
                    initial=0.0, op0=ALU.add, op1=ALU.bypass)

                # cross-row carries (prefix over 4 quadrant rows)
                def shift_add(dst, src_lo, src_hi, bounce):
                    # dst[hi] += dst[lo] via PSUM bounce (base-change copy)
                    nc.vector.tensor_copy(bounce[src_hi[0]:src_hi[1], :],
                                          dst[src_lo[0]:src_lo[1], :])
                    nc.vector.tensor_tensor(
                        dst[src_hi[0]:src_hi[1], :],
                        dst[src_hi[0]:src_hi[1], :],
                        bounce[src_hi[0]:src_hi[1], :], ALU.add)

                cu = sp.tile([P, 2], f32)   # col0: inclusive row totals
                nc.vector.tensor_copy(cu[:, 0:1], su[:, NC - 1:NC])
                bu = pmir[:, 0:1]
                shift_add(cu[:, 0:1], (0, 32), (32, 64), bu)
                shift_add(cu[:, 0:1], (32, 64), (64, 96), bu)
                shift_add(cu[:, 0:1], (64, 96), (96, 128), bu)
                nc.vector.tensor_tensor(
                    cu[:, 1:2], cu[:, 0:1], su[:, NC - 1:NC], ALU.subtract)
                cv = sp.tile([P, 2], f32)   # suffix carries (from higher rows)
                nc.vector.tensor_copy(cv[:, 0:1], sv[:, 0:1])
                bv = pmir[:, 1:2]
                shift_add(cv[:, 0:1], (96, 128), (64, 96), bv)
                shift_add(cv[:, 0:1], (64, 96), (32, 64), bv)
                shift_add(cv[:, 0:1], (32, 64), (0, 32), bv)
                nc.vector.tensor_tensor(
                    cv[:, 1:2], cv[:, 0:1], sv[:, 0:1], ALU.subtract)

                s1u = sp.tile([P, NC], f32)
                nc.vector.tensor_scalar(
                    s1u[:], su[:], scalar1=cu[:, 1:2], scalar2=None, op0=ALU.add)
                s2vi = sp.tile([P, NC], f32)
                nc.vector.tensor_scalar(
                    s2vi[:], sv[:], scalar1=cv[:, 1:2], scalar2=None, op0=ALU.add)
                nc.vector.tensor_tensor(s2vi[:], s2vi[:], v[:], ALU.subtract)

                feats = sp.tile([P, NC], f32)
                nc.vector.tensor_tensor(feats[:], v[:], s1u[:], ALU.mult)
                fb = sp.tile([P, NC], f32)
                nc.vector.tensor_tensor(fb[:], u[:], s2vi[:], ALU.mult)
                nc.vector.tensor_tensor(feats[:], feats[:], fb[:], ALU.add)
                if dbg:
                    nc.sync.dma_start(dbg_aps["d_u"][:], u[:])
                    nc.sync.dma_start(dbg_aps["d_s1u"][:], s1u[:])
                    nc.sync.dma_start(dbg_aps["d_s2v"][:], s2vi[:])
                    nc.sync.dma_start(dbg_aps["d_feats"][:], feats[:])

                if upto <= 5:
                    nc.sync.dma_start(out[:, 0:BS], feats[0:1, 0:BS])
                    continue
                # ======== phase 6: pack + sort #2 (inverse perm) ========
                key2 = sp.tile([P, NC], f32)
                nc.vector.tensor_scalar(
                    key2[:], feats[:], scalar1=FSCALE, scalar2=RC,
                    op0=ALU.mult, op1=ALU.add)
                nc.vector.tensor_scalar(
                    key2[:], key2[:], scalar1=RC, scalar2=None, op0=ALU.subtract)
                jhi = sp.tile([P, NC], f32)
                nc.vector.tensor_scalar_mul(jhi[:], jf[:], 2.0 ** 13)
                nc.vector.tensor_tensor(key2[:], key2[:], jhi[:], ALU.add)

                emit_sort(nc, FULL_OPS, 32, key2[:], tmp[:], pmir[:], pitch,
                          ppitch,
                          mirror_copy=lambda o, i: nc.scalar.copy(o, i),
                          cp_engines=[lambda o, i: nc.gpsimd.tensor_copy(o, i),
                                      lambda o, i: nc.scalar.copy(o, i)],
                          mir_pool=pp2)
                if dbg:
                    nc.sync.dma_start(dbg_aps["d_key2s"][:], key2[:])

                # decode feats in original order (int mask of low 13 bits)
                k2i = sp.tile([P, NC], mybir.dt.int32)
                nc.vector.tensor_copy(k2i[:], key2[:])
                q2i = sp.tile([P, NC], mybir.dt.int32)
                nc.vector.tensor_scalar(
                    q2i[:], k2i[:], scalar1=8191, scalar2=None,
                    op0=ALU.bitwise_and)
                fdec = sp.tile([P, NC], f32)
                nc.vector.tensor_copy(fdec[:], q2i[:])
                nc.vector.tensor_scalar(
                    fdec[:], fdec[:], scalar1=FDEC, scalar2=None, op0=ALU.mult)
                if dbg:
                    nc.sync.dma_start(dbg_aps["d_fdec"][:], fdec[:])

                if upto <= 6:
                    nc.sync.dma_start(out[:, 0:BS], fdec[0:1, 0:BS])
                    continue
                # ======== phase 7: weighted reduce; host sums partials ====
                # r=3 quadrant is at base 96 (illegal for PE rhs): copy to 64
                fx = sp.tile([P, NC], f32, name="fx")
                nc.scalar.copy(fx[64:64 + FL, :], fdec[96:96 + FL, :])
                contrib = sp.tile([1, B], f32)
                for r in range(NR):
                    rhs = (fdec[32 * r:32 * r + FL, :] if r < 3
                           else fx[64:64 + FL, :])
                    lb = 32 * r if r < 3 else 64
                    pcon = pp2.tile([1, NC], f32, tag="con", bufs=2,
                                    name=f"pcon{r}")
                    nc.tensor.matmul(
                        pcon[:], Wffl_sb[lb:lb + FL, :], rhs,
                        start=True, stop=True)
                    nc.vector.tensor_copy(
                        contrib[:, r * NC:(r + 1) * NC], pcon[:])
                if dbg:
                    nc.sync.dma_start(dbg_aps["d_contrib"][:], contrib[:])
                nc.sync.dma_start(out[:], contrib[:])
                osb = sp.tile([1, BS], f32)
                nc.vector.tensor_scalar(
                    osb[:], hWf_sb[:], scalar1=bf_sb[:1, :1], scalar2=None,
                    op0=ALU.add)
                nc.sync.dma_start(outh[:], osb[:])

    nc.compile()
    return nc


def _build_in_maps(inputs):
    x = np.asarray(inputs["x"], np.float32)
    W1 = np.asarray(inputs["W1"], np.float32)
    b1 = np.asarray(inputs["b1"], np.float32)
    W2 = np.asarray(inputs["W2"], np.float32)
    b2 = np.asarray(inputs["b2"], np.float32)
    T = np.asarray(inputs["T"], np.float32)
    Wf = np.asarray(inputs["Wf"], np.float32)
    bf = np.asarray(inputs["bf"], np.float32)

    wff_pad = np.zeros((FPAD, 1), np.float32)
    wff_pad[:F, 0] = Wf[H2:, 0]

    common = {
        "W1": np.ascontiguousarray(W1),
        "b1c": np.ascontiguousarray(b1.reshape(H1, 1)),
        "W2": np.ascontiguousarray(W2),
        "b2c": np.ascontiguousarray(b2.reshape(H2, 1)),
        "Tm": np.ascontiguousarray(T),
        "Wfh": np.ascontiguousarray(Wf[:H2].reshape(H2, 1)),
        "bfc": np.ascontiguousarray(bf.reshape(1, 1)),
        "ones1": np.ones((1, P), np.float32),
        "ones8": np.ones((NCORES, 1), np.float32),
    }
    in_maps = []
    for d in range(NCORES):
        m = dict(common)
        m["xT"] = np.ascontiguousarray(x[d * BS:(d + 1) * BS, :].T)
        m["Wffl"] = np.ascontiguousarray(wff_pad[d * FL:(d + 1) * FL])
        iot = np.broadcast_to(
            (d * BS + np.arange(BS, dtype=np.float32)) / 2048.0, (P, BS))
        m["iotas"] = np.ascontiguousarray(iot.astype(np.float32))
        in_maps.append(m)
    return in_maps


_NC_CACHE = None


def _get_program():
    global _NC_CACHE
    if _NC_CACHE is None:
        _NC_CACHE = build_program()
    return _NC_CACHE


def kernel(x, W1, b1, W2, b2, T, Wf, bf):
    nc = _get_program()
    in_maps = _build_in_maps(dict(
        x=x, W1=W1, b1=b1, W2=W2, b2=b2, T=T, Wf=Wf, bf=bf))
    res = run_bass_kernel_spmd(nc, in_maps, core_ids=list(range(NCORES)))
    total = np.zeros(B, np.float64)
    for d in range(NCORES):
        total += res.results[d]["out"].ravel().astype(np.float64)
        total[d * BS:(d + 1) * BS] += res.results[d]["outh"].ravel()
    return total.reshape(B, 1).astype(np.float32)



# revision 6
# speedup vs baseline: 1.8569x; 1.8569x over previous
"""Trainium2 Bass kernel for nn_Discriminator: MLP + sort-based minibatch
discrimination with gpsimd local_scatter un-permutation. Self-contained."""
import numpy as np
import ml_dtypes

N = 2048
NROWS = 4
NCOLS = 512


def stages(n=None):
    if n is None:
        n = N
    out = []
    p = 1
    while p < n:
        k = p
        while k >= 1:
            lefts = []
            j = k % p
            while j <= n - 1 - k:
                for i in range(0, min(k, n - j - k)):
                    x = i + j
                    if (x // (2 * p)) == ((x + k) // (2 * p)):
                        lefts.append(x)
                j += 2 * k
            out.append((p, k, np.array(sorted(lefts), dtype=np.int64)))
            k //= 2
        p *= 2
    return out


def runs_of(xs):
    """Compress sorted ints into <=3-level pattern (start, L, s1, c1, s2, c2)."""
    xs = np.asarray(xs)
    if len(xs) == 0:
        return None
    breaks = np.where(np.diff(xs) != 1)[0]
    starts_i = np.concatenate([[0], breaks + 1])
    ends_i = np.concatenate([breaks, [len(xs) - 1]])
    run_starts = xs[starts_i]
    run_lens = ends_i - starts_i + 1
    if not np.all(run_lens == run_lens[0]):
        return None
    L = int(run_lens[0])
    if len(run_starts) == 1:
        return (int(run_starts[0]), L, 0, 1, 0, 1)
    d = np.diff(run_starts)
    if np.all(d == d[0]):
        return (int(run_starts[0]), L, int(d[0]), len(run_starts), 0, 1)
    s1 = d[0]
    c1 = 1
    while c1 < len(d) and d[c1 - 1] == s1:
        c1 += 1
    group = c1
    if len(run_starts) % group != 0:
        return None
    rs = run_starts.reshape(-1, group)
    inner = np.diff(rs, axis=1)
    starts2 = rs[:, 0]
    d2 = np.diff(starts2)
    if inner.size and not np.all(inner == s1):
        return None
    if len(d2) and not np.all(d2 == d2[0]):
        return None
    return (int(run_starts[0]), L, int(s1), group,
            int(d2[0]) if len(d2) else 0, len(starts2))


def emit_ops():
    """Returns list of (p, k, [ops]); op = (r0, nrows, drow, colpat, colB0)."""
    all_stages = []
    for (p, k, lefts) in stages():
        ops = []
        rows = lefts // NCOLS
        cols = lefts % NCOLS
        drows = (lefts + k) // NCOLS - rows
        for dr in np.unique(drows):
            sel = drows == dr
            rset = np.unique(rows[sel])
            cset = np.unique(cols[sel])
            assert sel.sum() == len(rset) * len(cset), (p, k, dr)
            for r in rset:
                cc = np.sort(cols[sel & (rows == r)])
                assert np.array_equal(cc, cset), (p, k, dr, r)
            colpat = runs_of(cset)
            assert colpat is not None, (p, k, dr, cset[:20])
            rpat = runs_of(rset)
            assert rpat is not None, (p, k, dr, rset)
            (r0, Lr, sr1, cr1, sr2, cr2) = rpat
            assert sr2 == 0 and cr2 == 1, (p, k, dr, rpat)
            colB0 = int((cset[0] + k) % NCOLS)
            for g in range(cr1):
                rstart = r0 + g * sr1
                ops.append((int(rstart), int(Lr), int(dr), colpat, colB0))
        all_stages.append((p, int(k), ops))
    return all_stages


def _row_chunks(a_base, b_base, nr):
    allowed = {0: 4, 1: 1, 2: 2, 3: 1}
    out = []
    off = 0
    while off < nr:
        c = min(allowed[(a_base + off) % 4], allowed[(b_base + off) % 4], nr - off)
        out.append((off, c))
        off += c
    return out


def legalize(all_stages):
    out = []
    for (p, k, ops) in all_stages:
        nops = []
        for (r0, nr, dr, colpat, colB0) in ops:
            for (off, c) in _row_chunks(r0, r0 + dr, nr):
                nops.append((r0 + off, c, dr, colpat, colB0))
        out.append((p, k, nops))
    return out


def colpat_idx(colpat):
    (c0, L, s1, c1, s2, c2) = colpat
    return (c0 + np.arange(c2)[:, None, None] * s2
            + np.arange(c1)[None, :, None] * s1
            + np.arange(L)[None, None, :]).ravel()


def runs_multi(xs, max_groups=6):
    xs = np.asarray(xs)
    if len(xs) == 0:
        return []
    r = runs_of(xs)
    if r is not None:
        return [r]
    breaks = np.where(np.diff(xs) != 1)[0]
    starts_i = np.concatenate([[0], breaks + 1])
    ends_i = np.concatenate([breaks, [len(xs) - 1]])
    run_starts = xs[starts_i]
    run_lens = ends_i - starts_i + 1
    out = []
    for L in np.unique(run_lens):
        sel = run_lens == L
        rs = run_starts[sel]
        d = np.diff(rs)
        if len(d) == 0 or np.all(d == d[0]):
            out.append((int(rs[0]), int(L), int(d[0]) if len(d) else 0,
                        len(rs), 0, 1))
        else:
            for s in rs:
                out.append((int(s), int(L), 0, 1, 0, 1))
    return out


def emit_pingpong():
    out = []
    for (p, k, ops) in legalize(emit_ops()):
        touched = np.zeros((NROWS, NCOLS), dtype=bool)
        for (r0, nr, dr, colpat, colB0) in ops:
            ia = colpat_idx(colpat)
            ib = ia + (colB0 - colpat[0])
            for rr in range(r0, r0 + nr):
                touched[rr, ia] = True
                touched[rr + dr, ib] = True
        cp_ops = []
        r = 0
        while r < NROWS:
            mask = ~touched[r]
            r2 = r + 1
            while r2 < NROWS and np.array_equal(~touched[r2], mask):
                r2 += 1
            cols = np.where(mask)[0]
            if len(cols):
                for pat in runs_multi(cols):
                    off = 0
                    nr_ = r2 - r
                    allowed = {0: 4, 1: 1, 2: 2, 3: 1}
                    while off < nr_:
                        c = min(allowed[(r + off) % 4], nr_ - off)
                        cp_ops.append((r + off, c, pat))
                        off += c
            r = r2
        out.append((p, k, ops, cp_ops))
    return out


def _split_colpat(colpat, max_free=288):
    (c0, L, s1, c1, s2, c2) = colpat
    free = L * c1 * c2
    if free <= max_free:
        return [(0, colpat)]
    if c2 > 1:
        h = c2 // 2
        a = (c0, L, s1, c1, s2, h)
        b = (c0 + h * s2, L, s1, c1, s2, c2 - h)
        return [(d, p) for d0, pp_ in [(0, a), (h * s2, b)]
                for d, p in [(d0 + dd, p2) for dd, p2 in _split_colpat(
                    (pp_[0], pp_[1], pp_[2], pp_[3], pp_[4], pp_[5]), max_free)]]
    if c1 > 1:
        h = c1 // 2
        a = (c0, L, s1, h, 0, 1)
        b = (c0 + h * s1, L, s1, c1 - h, 0, 1)
        out = []
        for base, pat in [(0, a), (h * s1, b)]:
            out.extend(_split_colpat(pat, max_free))
        return out
    h = L // 2
    a = (c0, h, 0, 1, 0, 1)
    b = (c0 + h, L - h, 0, 1, 0, 1)
    return _split_colpat(a, max_free) + _split_colpat(b, max_free)


def drain_split(stages_pp, max_free=288):
    out = []
    for (p, k, cmp_ops, cp_ops) in stages_pp:
        nc_ops = []
        for (r0, nr, dr, colpat, colB0) in cmp_ops:
            for (_, pat) in _split_colpat(colpat, max_free):
                nb0 = colB0 + (pat[0] - colpat[0])
                nc_ops.append((r0, nr, dr, pat, nb0))
        ncp_ops = []
        for (r0, nr, pat) in cp_ops:
            for (_, p2) in _split_colpat(pat, max_free):
                ncp_ops.append((r0, nr, p2))
        out.append((p, k, nc_ops, ncp_ops))
    return out


def gen_pingpong(n, nrows, ncols, p_min=1, max_free=288):
    global N, NROWS, NCOLS
    oldN, oldR, oldC = N, NROWS, NCOLS
    N, NROWS, NCOLS = n, nrows, ncols
    try:
        full = emit_pingpong()
        filt = [(p, k, c, cp) for (p, k, c, cp) in full if p >= p_min]
        return drain_split(filt, max_free)
    finally:
        N, NROWS, NCOLS = oldN, oldR, oldC


import bass_rust
import concourse.bacc as bacc
import concourse.mybir as mybir
from concourse import tile
from concourse.bass_utils import run_bass_kernel_spmd


B, D, H1, H2, F = 2048, 3072, 512, 256, 100
NCORES = 8
BS = B // NCORES            # 256 rows per core
LEAK = 0.2
P = 128
FL = 13                     # features per core (8*13 = 104 >= 100)
FPAD = NCORES * FL          # 104
NR, NC = 4, 512
RC = 2.0 ** 23              # rounding constant
QLEV = 8190.0
MRANGE = 16.0
QSCALE = QLEV / (2 * MRANGE)
DQ = (2 * MRANGE) / QLEV

f32 = mybir.dt.float32
f16 = mybir.dt.float16
bf16 = mybir.dt.bfloat16
i16 = mybir.dt.int16
i32 = mybir.dt.int32
AF = mybir.ActivationFunctionType
ALU = mybir.AluOpType

KD, K1, K2 = D // P, H1 // P, H2 // P     # 24, 4, 2
NCHUNK = 4                                 # DMA chunks for W1/x
KCH = KD // NCHUNK                         # 6 k-blocks per chunk


def sap(t_ap, pitch, pstart, pcount, coff, colpat):
    """Strided AP view: partitions [pstart, pstart+pcount), free pattern
    colpat=(c0,L,s1,c1,s2,c2) shifted to coff."""
    (c0, L, s1, c1, s2, c2) = colpat
    dims = [(pitch, pcount)]
    if c2 > 1:
        dims.append((s2, c2))
    if c1 > 1:
        dims.append((s1, c1))
    dims.append((1, L))
    a = t_ap.copy()
    a.ap = bass_rust.VecI64Pair(dims)
    a.offset = pstart * pitch + coff
    return a


SRC_OPS = gen_pingpong(256, 1, 256)
MRG_OPS = gen_pingpong(2048, 4, 512, p_min=256)


def emit_sort(nc, ops_table, rowpart, buf, tmp, pitch,
              cp_engines, mir_pool=None):
    """Ping-pong odd-even merge sort between `buf` and `tmp` (DVE min/max).
    Cross-row compares read the B operand through an SBUF mirror copied by
    ACT/Pool (partition-shifted copies are legal in both directions)."""
    bufs = [buf, tmp]
    ci = 0
    mi = 0
    for si, (p, k, cmp_ops, cp_ops) in enumerate(ops_table):
        cur = bufs[si % 2]
        nxt = bufs[(si + 1) % 2]
        for (r0, nr, dr, colpat, colB0) in cmp_ops:
            pa, pb = rowpart * r0, rowpart * (r0 + dr)
            npart = rowpart * nr
            a_in = sap(cur, pitch, pa, npart, colpat[0], colpat)
            a_out = sap(nxt, pitch, pa, npart, colpat[0], colpat)
            b_out = sap(nxt, pitch, pb, npart, colB0, colpat)
            if dr == 0:
                b_in = sap(cur, pitch, pb, npart, colB0, colpat)
            else:
                b_cur = sap(cur, pitch, pb, npart, colB0, colpat)
                mt = mir_pool.tile([128, 512], f32, tag="mir", bufs=4,
                                   name="mirt")
                b_in = sap(mt[:], mt[:].ap[0][0], pa, npart,
                           colpat[0], colpat)
                if mi % 2 == 0:
                    nc.scalar.copy(b_in, b_cur)
                else:
                    nc.gpsimd.tensor_copy(b_in, b_cur)
                mi += 1
            nc.vector.tensor_tensor(a_out, a_in, b_in, ALU.min)
            nc.vector.tensor_tensor(b_out, a_in, b_in, ALU.max)
        for (r0, nr, pat) in cp_ops:
            pa = rowpart * r0
            npart = rowpart * nr
            c_in = sap(cur, pitch, pa, npart, pat[0], pat)
            c_out = sap(nxt, pitch, pa, npart, pat[0], pat)
            cp_engines[ci % len(cp_engines)](c_out, c_in)
            ci += 1


def build_program():
    nc = bacc.Bacc(
        "TRN2", target_bir_lowering=False, debug=False, num_devices=NCORES)

    xTp = nc.dram_tensor("xTp", [P, KD * BS], bf16, kind="ExternalInput").ap()
    W1p = nc.dram_tensor("W1p", [P, KD * H1], bf16, kind="ExternalInput").ap()
    W2p = nc.dram_tensor("W2p", [P, K1 * H2], bf16, kind="ExternalInput").ap()
    Tp = nc.dram_tensor("Tp", [P, K2 * F], bf16, kind="ExternalInput").ap()
    Wfhp = nc.dram_tensor("Wfhp", [P, K2], bf16, kind="ExternalInput").ap()
    b1c = nc.dram_tensor("b1c", [P, K1], f32, kind="ExternalInput").ap()
    b2c = nc.dram_tensor("b2c", [P, K2], f32, kind="ExternalInput").ap()
    bfc = nc.dram_tensor("bfc", [1, 1], f32, kind="ExternalInput").ap()
    iotas = nc.dram_tensor("iotas", [P, BS], f32, kind="ExternalInput").ap()
    w16d = nc.dram_tensor("w16", [P, 1], f16, kind="ExternalInput").ap()
    pmaskd = nc.dram_tensor("pmask", [P, 1], i16, kind="ExternalInput").ap()
    lmA = nc.dram_tensor("lmA", [P, P], f32, kind="ExternalInput").ap()
    lmB = nc.dram_tensor("lmB", [P, P], f32, kind="ExternalInput").ap()
    out = nc.dram_tensor("out", [1, B], f32, kind="ExternalOutput").ap()
    outh = nc.dram_tensor("outh", [1, BS], f32, kind="ExternalOutput").ap()

    with tile.TileContext(nc) as tc:
        with (
            tc.tile_pool(name="persist", bufs=1) as pers,
            tc.tile_pool(name="dram", bufs=1, space="DRAM") as dpool,
        ):
            # ---- small persistent loads ----
            bf_sb = pers.tile([1, 1], f32)
            nc.sync.dma_start(bf_sb[:], bfc)
            iota_sb = pers.tile([P, BS], f32)
            nc.sync.dma_start(iota_sb[:], iotas)
            w16_sb = pers.tile([P, 1], f16)
            nc.sync.dma_start(w16_sb[:], w16d)
            pmask_sb = pers.tile([P, 1], i16)
            nc.sync.dma_start(pmask_sb[:], pmaskd)
            lmA_sb = pers.tile([P, P], f32)
            nc.sync.dma_start(lmA_sb[:], lmA)
            lmB_sb = pers.tile([P, P], f32)
            nc.sync.dma_start(lmB_sb[:], lmB)
            b1_sb = pers.tile([P, K1], f32)
            nc.sync.dma_start(b1_sb[:], b1c)
            b2_sb = pers.tile([P, K2], f32)
            nc.sync.dma_start(b2_sb[:], b2c)

            hWf_sb = pers.tile([1, BS], f32)
            mT_loc = pers.tile([F, BS], f32)

            # ======== phase 1: MLP (bf16 weights/activations) ========
            with (
                tc.tile_pool(name="mlp", bufs=1) as mp,
                tc.tile_pool(name="psum_mm", bufs=1, space="PSUM") as pmm,
            ):
                W2_sb = mp.tile([P, K1 * H2], bf16)
                nc.sync.dma_start(W2_sb[:], W2p)
                T_sb = mp.tile([P, K2 * F], bf16)
                nc.sync.dma_start(T_sb[:], Tp)
                Wfh_sb = mp.tile([P, K2], bf16)
                nc.sync.dma_start(Wfh_sb[:], Wfhp)

                xT_sb = mp.tile([P, KD * BS], bf16)
                W1_sb = mp.tile([P, KD * H1], bf16)
                for c in range(NCHUNK):
                    lo = c * KCH
                    nc.sync.dma_start(
                        W1_sb[:, lo * H1:(lo + KCH) * H1],
                        W1p[:, lo * H1:(lo + KCH) * H1])
                    nc.sync.dma_start(
                        xT_sb[:, lo * BS:(lo + KCH) * BS],
                        xTp[:, lo * BS:(lo + KCH) * BS])

                pt1 = [pmm.tile([P, BS], f32, name=f"pt1_{mb}")
                       for mb in range(K1)]
                for k in range(KD):
                    for mb in range(K1):
                        nc.tensor.matmul(
                            pt1[mb][:],
                            W1_sb[:, k * H1 + mb * P: k * H1 + (mb + 1) * P],
                            xT_sb[:, k * BS:(k + 1) * BS],
                            start=(k == 0), stop=(k == KD - 1))
                h1T = [mp.tile([P, BS], bf16, name=f"h1T{m}") for m in range(K1)]
                for mb in range(K1):
                    s1 = mp.tile([P, BS], f32, tag="stmp", bufs=2,
                                 name=f"s1_{mb}")
                    nc.scalar.activation(
                        s1[:], pt1[mb][:], AF.Identity, bias=b1_sb[:, mb:mb + 1])
                    nc.vector.scalar_tensor_tensor(
                        h1T[mb][:], s1[:], LEAK, s1[:], op0=ALU.mult,
                        op1=ALU.max)

                pt2 = [pmm.tile([P, BS], f32, name=f"pt2_{mb}")
                       for mb in range(K2)]
                for k in range(K1):
                    for mb in range(K2):
                        nc.tensor.matmul(
                            pt2[mb][:],
                            W2_sb[:, k * H2 + mb * P: k * H2 + (mb + 1) * P],
                            h1T[k][:],
                            start=(k == 0), stop=(k == K1 - 1))
                h2T = [mp.tile([P, BS], bf16, name=f"h2T{m}") for m in range(K2)]
                for mb in range(K2):
                    s2 = mp.tile([P, BS], f32, tag="stmp", bufs=2,
                                 name=f"s2_{mb}")
                    nc.scalar.activation(
                        s2[:], pt2[mb][:], AF.Identity, bias=b2_sb[:, mb:mb + 1])
                    nc.vector.scalar_tensor_tensor(
                        h2T[mb][:], s2[:], LEAK, s2[:], op0=ALU.mult,
                        op1=ALU.max)

                pt_m = pmm.tile([F, BS], f32, name="ptm")
                for k in range(K2):
                    nc.tensor.matmul(
                        pt_m[:], T_sb[:, k * F:(k + 1) * F], h2T[k][:],
                        start=(k == 0), stop=(k == K2 - 1))
                nc.scalar.copy(mT_loc[:], pt_m[:])

                ph = pmm.tile([1, BS], f32, name="ph")
                for k in range(K2):
                    nc.tensor.matmul(
                        ph[:], Wfh_sb[:, k:k + 1], h2T[k][:],
                        start=(k == 0), stop=(k == K2 - 1))
                nc.vector.tensor_copy(hWf_sb[:], ph[:])

            # ======== phase 2: quantize + pack + local sort ========
            skey = pers.tile([P, BS], f32)
            sktmp = pers.tile([P, BS], f32)
            nc.vector.memset(skey[:], 0.0)
            nc.vector.tensor_scalar(
                skey[:F, :], mT_loc[:], scalar1=MRANGE, scalar2=QSCALE,
                op0=ALU.add, op1=ALU.mult)
            nc.vector.tensor_scalar(
                skey[:F, :], skey[:F, :], scalar1=RC, scalar2=RC,
                op0=ALU.add, op1=ALU.subtract)
            nc.vector.tensor_scalar(
                skey[:F, :], skey[:F, :], scalar1=8191.0, scalar2=0.0,
                op0=ALU.min, op1=ALU.max)
            nc.vector.tensor_tensor(skey[:F, :], skey[:F, :], iota_sb[:F, :],
                                    ALU.add)
            spitch = skey[:].ap[0][0]
            emit_sort(nc, SRC_OPS, P, skey[:], sktmp[:], spitch,
                      [lambda o, i: nc.scalar.copy(o, i),
                       lambda o, i: nc.gpsimd.tensor_copy(o, i)])

            # ======== phase 3: AllToAll ========
            a2a_in = dpool.tile([FPAD, BS], f32)
            a2a_out = dpool.tile([FPAD, BS], f32)
            nc.sync.dma_start(a2a_in[:F, :], skey[:F, :])
            nc.sync.dma_start(a2a_in[F:FPAD, :], skey[:FPAD - F, :])
            nc.gpsimd.collective_compute(
                "AllToAll", ALU.bypass,
                replica_groups=[list(range(NCORES))],
                ins=[a2a_in.opt()], outs=[a2a_out.opt()])

            key = pers.tile([P, NC], f32)
            nc.vector.memset(key[:], 0.0)
            for e in range(NCORES):
                r, half = e // 2, e % 2
                nc.sync.dma_start(
                    key[32 * r:32 * r + FL, half * BS:(half + 1) * BS],
                    a2a_out[e * FL:(e + 1) * FL, :])

            # ======== phase 4: merge (30 stages) + scan + unsort ========
            pitch = key[:].ap[0][0]
            with (
                tc.tile_pool(name="sortp", bufs=1) as sp,
                tc.tile_pool(name="psum2", bufs=1, space="PSUM") as pp2,
            ):
                tmp = sp.tile([P, NC], f32)
                emit_sort(nc, MRG_OPS, 32, key[:], tmp[:], pitch,
                          cp_engines=[lambda o, i: nc.scalar.copy(o, i),
                                      lambda o, i: nc.gpsimd.tensor_copy(o, i)],
                          mir_pool=sp)

                # ---- scan phase: split key = g + j/2048 ----
                kq = sp.tile([P, NC], f32)
                nc.vector.tensor_scalar_mul(kq[:], key[:], 2048.0)
                ki = sp.tile([P, NC], i32)
                nc.vector.tensor_copy(ki[:], kq[:])
                ji = sp.tile([P, NC], i32)
                nc.vector.tensor_scalar(
                    ji[:], ki[:], scalar1=2047, scalar2=None,
                    op0=ALU.bitwise_and)
                ji16 = sp.tile([P, NC], i16)
                nc.gpsimd.tensor_copy(ji16[:], ji[:])
                gi = sp.tile([P, NC], i32)
                nc.vector.tensor_scalar(
                    gi[:], ki[:], scalar1=-2048, scalar2=None,
                    op0=ALU.bitwise_and)
                g2k = sp.tile([P, NC], f32)
                nc.vector.tensor_copy(g2k[:], gi[:])
                bneg = sp.tile([P, 1], f32)
                nc.vector.memset(bneg[:], -MRANGE)
                bpos = sp.tile([P, 1], f32)
                nc.vector.memset(bpos[:], MRANGE)
                u = sp.tile([P, NC], f32)
                nc.scalar.activation(
                    u[:], g2k[:], AF.Exp, bias=bneg[:], scale=DQ / 2048.0)
                v = sp.tile([P, NC], f32)
                nc.scalar.activation(
                    v[:], g2k[:], AF.Exp, bias=bpos[:], scale=-DQ / 2048.0)

                su = sp.tile([P, NC], f32)
                nc.vector.tensor_tensor_scan(
                    su[:], u[:], u[:], initial=0.0, op0=ALU.add,
                    op1=ALU.bypass)
                sv = sp.tile([P, NC], f32)
                nc.vector.tensor_tensor_scan(
                    sv[:, NC - 1::-1], v[:, NC - 1::-1], v[:, NC - 1::-1],
                    initial=0.0, op0=ALU.add, op1=ALU.bypass)

                # cross-quadrant carries via masked prefix matmuls
                pcu = pp2.tile([P, 2], f32, name="pcu")
                nc.tensor.matmul(pcu[:, 0:1], lmA_sb[:], su[:, NC - 1:NC],
                                 start=True, stop=True)
                nc.tensor.matmul(pcu[:, 1:2], lmB_sb[:], sv[:, 0:1],
                                 start=True, stop=True)
                carr = sp.tile([P, 2], f32)
                nc.vector.tensor_copy(carr[:], pcu[:])

                s1u = sp.tile([P, NC], f32)
                nc.vector.tensor_scalar(
                    s1u[:], su[:], scalar1=carr[:, 0:1], scalar2=None,
                    op0=ALU.add)
                s2vi = sp.tile([P, NC], f32)
                nc.vector.tensor_scalar(
                    s2vi[:], sv[:], scalar1=carr[:, 1:2], scalar2=None,
                    op0=ALU.add)
                nc.vector.tensor_tensor(s2vi[:], s2vi[:], v[:], ALU.subtract)

                fa = sp.tile([P, NC], f32)
                nc.vector.tensor_tensor(fa[:], v[:], s1u[:], ALU.mult)
                fb = sp.tile([P, NC], f32)
                nc.gpsimd.tensor_tensor(fb[:], u[:], s2vi[:], ALU.mult)
                feats16 = sp.tile([P, NC], f16)
                nc.vector.tensor_tensor(feats16[:], fa[:], fb[:], ALU.add)

                # ---- unsort via local_scatter (j < 1024 | j >= 1024) ----
                tt = sp.tile([P, NC], i16)
                nc.vector.tensor_scalar(
                    tt[:], ji16[:], scalar1=pmask_sb[:], scalar2=None,
                    op0=ALU.bitwise_or)
                neg1 = sp.tile([P, NC], i16)
                nc.gpsimd.memset(neg1[:], -1)
                m0 = sp.tile([P, NC], i16)
                nc.vector.tensor_scalar(
                    m0[:], tt[:], scalar1=1023, scalar2=None, op0=ALU.is_le)
                idx0 = sp.tile([P, NC], i16)
                nc.vector.select(idx0[:], m0[:], tt[:], neg1[:])
                t1 = sp.tile([P, NC], i16)
                nc.vector.tensor_scalar(
                    t1[:], tt[:], scalar1=1024, scalar2=None, op0=ALU.subtract)
                idx1 = sp.tile([P, NC], i16)
                nc.vector.select(idx1[:], m0[:], neg1[:], t1[:])

                dst0 = sp.tile([P, 2 * NC], f16)
                dst1 = sp.tile([P, 2 * NC], f16)
                nc.gpsimd.local_scatter(
                    dst0[:], feats16[:], idx0[:], channels=P,
                    num_elems=2 * NC, num_idxs=NC)
                nc.gpsimd.local_scatter(
                    dst1[:], feats16[:], idx1[:], channels=P,
                    num_elems=2 * NC, num_idxs=NC)

                contrib = sp.tile([1, B], f32)
                for h, dst in ((0, dst0), (1, dst1)):
                    for s in range(2):
                        pc = pp2.tile([1, NC], f32, tag="pc", bufs=1,
                                      name=f"pc{h}{s}")
                        nc.tensor.matmul(
                            pc[:], w16_sb[:], dst[:, s * NC:(s + 1) * NC],
                            start=True, stop=True)
                        nc.vector.tensor_copy(
                            contrib[:, h * 1024 + s * NC:
                                    h * 1024 + (s + 1) * NC], pc[:])
                nc.sync.dma_start(out[:], contrib[:])
                osb = sp.tile([1, BS], f32)
                nc.vector.tensor_scalar(
                    osb[:], hWf_sb[:], scalar1=bf_sb[:1, :1], scalar2=None,
                    op0=ALU.add)
                nc.sync.dma_start(outh[:], osb[:])

    nc.compile()
    return nc


def _build_in_maps(inputs):
    x = np.asarray(inputs["x"], np.float32)
    W1 = np.asarray(inputs["W1"], np.float32)
    b1 = np.asarray(inputs["b1"], np.float32)
    W2 = np.asarray(inputs["W2"], np.float32)
    b2 = np.asarray(inputs["b2"], np.float32)
    T = np.asarray(inputs["T"], np.float32)
    Wf = np.asarray(inputs["Wf"], np.float32)
    bf = np.asarray(inputs["bf"], np.float32)

    bfl = ml_dtypes.bfloat16
    W1p = np.ascontiguousarray(
        W1.reshape(KD, P, H1).transpose(1, 0, 2).reshape(P, KD * H1)
    ).astype(bfl)
    W2p = np.ascontiguousarray(
        W2.reshape(K1, P, H2).transpose(1, 0, 2).reshape(P, K1 * H2)
    ).astype(bfl)
    Tp = np.ascontiguousarray(
        T.reshape(K2, P, F).transpose(1, 0, 2).reshape(P, K2 * F)
    ).astype(bfl)
    Wfhp = np.ascontiguousarray(
        Wf[:H2].reshape(K2, P).T).astype(bfl)
    b1p = np.ascontiguousarray(b1.reshape(K1, P).T)
    b2p = np.ascontiguousarray(b2.reshape(K2, P).T)

    wff = Wf[H2:, 0]
    wff_pad = np.zeros(FPAD, np.float32)
    wff_pad[:F] = wff

    lmaskA = np.zeros((P, P), np.float32)
    lmaskB = np.zeros((P, P), np.float32)
    for k in range(P):
        for m in range(P):
            if k % 32 == m % 32:
                if k // 32 < m // 32:
                    lmaskA[k, m] = 1.0
                elif k // 32 > m // 32:
                    lmaskB[k, m] = 1.0

    pmask = np.full((P, 1), -1, np.int16)
    for r in range(NR):
        pmask[32 * r:32 * r + FL] = 0

    common = {
        "W1p": W1p, "W2p": W2p, "Tp": Tp, "Wfhp": Wfhp,
        "b1c": b1p, "b2c": b2p,
        "bfc": np.ascontiguousarray(bf.reshape(1, 1)),
        "lmA": lmaskA, "lmB": lmaskB, "pmask": pmask,
    }
    in_maps = []
    for d in range(NCORES):
        m = dict(common)
        xT = x[d * BS:(d + 1) * BS, :].T
        m["xTp"] = np.ascontiguousarray(
            xT.reshape(KD, P, BS).transpose(1, 0, 2).reshape(P, KD * BS)
        ).astype(bfl)
        w16 = np.zeros((P, 1), np.float16)
        for r in range(NR):
            w16[32 * r:32 * r + FL, 0] = wff_pad[d * FL:(d + 1) * FL]
        m["w16"] = w16
        iot = np.broadcast_to(
            (d * BS + np.arange(BS, dtype=np.float32)) / 2048.0, (P, BS))
        m["iotas"] = np.ascontiguousarray(iot.astype(np.float32))
        in_maps.append(m)
    return in_maps


_NC_CACHE = None


def _get_program():
    global _NC_CACHE
    if _NC_CACHE is None:
        _NC_CACHE = build_program()
    return _NC_CACHE


def kernel(x, W1, b1, W2, b2, T, Wf, bf):
    nc = _get_program()
    in_maps = _build_in_maps(dict(
        x=x, W1=W1, b1=b1, W2=W2, b2=b2, T=T, Wf=Wf, bf=bf))
    res = run_bass_kernel_spmd(nc, in_maps, core_ids=list(range(NCORES)))
    total = np.zeros(B, np.float64)
    for d in range(NCORES):
        total += res.results[d]["out"].ravel().astype(np.float64)
        total[d * BS:(d + 1) * BS] += res.results[d]["outh"].ravel()
    return total.reshape(B, 1).astype(np.float32)


# revision 14
# speedup vs baseline: 1.9472x; 1.0486x over previous
"""Trainium2 Bass kernel for nn_Discriminator: MLP + sort-based minibatch
discrimination with gpsimd local_scatter un-permutation. Self-contained."""
import numpy as np
import ml_dtypes

N = 2048
NROWS = 4
NCOLS = 512


def stages(n=None):
    if n is None:
        n = N
    out = []
    p = 1
    while p < n:
        k = p
        while k >= 1:
            lefts = []
            j = k % p
            while j <= n - 1 - k:
                for i in range(0, min(k, n - j - k)):
                    x = i + j
                    if (x // (2 * p)) == ((x + k) // (2 * p)):
                        lefts.append(x)
                j += 2 * k
            out.append((p, k, np.array(sorted(lefts), dtype=np.int64)))
            k //= 2
        p *= 2
    return out


def runs_of(xs):
    """Compress sorted ints into <=3-level pattern (start, L, s1, c1, s2, c2)."""
    xs = np.asarray(xs)
    if len(xs) == 0:
        return None
    breaks = np.where(np.diff(xs) != 1)[0]
    starts_i = np.concatenate([[0], breaks + 1])
    ends_i = np.concatenate([breaks, [len(xs) - 1]])
    run_starts = xs[starts_i]
    run_lens = ends_i - starts_i + 1
    if not np.all(run_lens == run_lens[0]):
        return None
    L = int(run_lens[0])
    if len(run_starts) == 1:
        return (int(run_starts[0]), L, 0, 1, 0, 1)
    d = np.diff(run_starts)
    if np.all(d == d[0]):
        return (int(run_starts[0]), L, int(d[0]), len(run_starts), 0, 1)
    s1 = d[0]
    c1 = 1
    while c1 < len(d) and d[c1 - 1] == s1:
        c1 += 1
    group = c1
    if len(run_starts) % group != 0:
        return None
    rs = run_starts.reshape(-1, group)
    inner = np.diff(rs, axis=1)
    starts2 = rs[:, 0]
    d2 = np.diff(starts2)
    if inner.size and not np.all(inner == s1):
        return None
    if len(d2) and not np.all(d2 == d2[0]):
        return None
    return (int(run_starts[0]), L, int(s1), group,
            int(d2[0]) if len(d2) else 0, len(starts2))


def emit_ops():
    """Returns list of (p, k, [ops]); op = (r0, nrows, drow, colpat, colB0)."""
    all_stages = []
    for (p, k, lefts) in stages():
        ops = []
        rows = lefts // NCOLS
        cols = lefts % NCOLS
        drows = (lefts + k) // NCOLS - rows
        for dr in np.unique(drows):
            sel = drows == dr
            rset = np.unique(rows[sel])
            cset = np.unique(cols[sel])
            assert sel.sum() == len(rset) * len(cset), (p, k, dr)
            for r in rset:
                cc = np.sort(cols[sel & (rows == r)])
                assert np.array_equal(cc, cset), (p, k, dr, r)
            colpat = runs_of(cset)
            assert colpat is not None, (p, k, dr, cset[:20])
            rpat = runs_of(rset)
            assert rpat is not None, (p, k, dr, rset)
            (r0, Lr, sr1, cr1, sr2, cr2) = rpat
            assert sr2 == 0 and cr2 == 1, (p, k, dr, rpat)
            colB0 = int((cset[0] + k) % NCOLS)
            for g in range(cr1):
                rstart = r0 + g * sr1
                ops.append((int(rstart), int(Lr), int(dr), colpat, colB0))
        all_stages.append((p, int(k), ops))
    return all_stages


def _row_chunks(a_base, b_base, nr):
    allowed = {0: 4, 1: 1, 2: 2, 3: 1}
    out = []
    off = 0
    while off < nr:
        c = min(allowed[(a_base + off) % 4], allowed[(b_base + off) % 4], nr - off)
        out.append((off, c))
        off += c
    return out


def legalize(all_stages):
    out = []
    for (p, k, ops) in all_stages:
        nops = []
        for (r0, nr, dr, colpat, colB0) in ops:
            for (off, c) in _row_chunks(r0, r0 + dr, nr):
                nops.append((r0 + off, c, dr, colpat, colB0))
        out.append((p, k, nops))
    return out


def colpat_idx(colpat):
    (c0, L, s1, c1, s2, c2) = colpat
    return (c0 + np.arange(c2)[:, None, None] * s2
            + np.arange(c1)[None, :, None] * s1
            + np.arange(L)[None, None, :]).ravel()


def runs_multi(xs, max_groups=6):
    xs = np.asarray(xs)
    if len(xs) == 0:
        return []
    r = runs_of(xs)
    if r is not None:
        return [r]
    breaks = np.where(np.diff(xs) != 1)[0]
    starts_i = np.concatenate([[0], breaks + 1])
    ends_i = np.concatenate([breaks, [len(xs) - 1]])
    run_starts = xs[starts_i]
    run_lens = ends_i - starts_i + 1
    out = []
    for L in np.unique(run_lens):
        sel = run_lens == L
        rs = run_starts[sel]
        d = np.diff(rs)
        if len(d) == 0 or np.all(d == d[0]):
            out.append((int(rs[0]), int(L), int(d[0]) if len(d) else 0,
                        len(rs), 0, 1))
        else:
            for s in rs:
                out.append((int(s), int(L), 0, 1, 0, 1))
    return out


def emit_pingpong():
    out = []
    for (p, k, ops) in legalize(emit_ops()):
        touched = np.zeros((NROWS, NCOLS), dtype=bool)
        for (r0, nr, dr, colpat, colB0) in ops:
            ia = colpat_idx(colpat)
            ib = ia + (colB0 - colpat[0])
            for rr in range(r0, r0 + nr):
                touched[rr, ia] = True
                touched[rr + dr, ib] = True
        cp_ops = []
        r = 0
        while r < NROWS:
            mask = ~touched[r]
            r2 = r + 1
            while r2 < NROWS and np.array_equal(~touched[r2], mask):
                r2 += 1
            cols = np.where(mask)[0]
            if len(cols):
                for pat in runs_multi(cols):
                    off = 0
                    nr_ = r2 - r
                    allowed = {0: 4, 1: 1, 2: 2, 3: 1}
                    while off < nr_:
                        c = min(allowed[(r + off) % 4], nr_ - off)
                        cp_ops.append((r + off, c, pat))
                        off += c
            r = r2
        out.append((p, k, ops, cp_ops))
    return out


def _split_colpat(colpat, max_free=288):
    (c0, L, s1, c1, s2, c2) = colpat
    free = L * c1 * c2
    if free <= max_free:
        return [(0, colpat)]
    if c2 > 1:
        h = c2 // 2
        a = (c0, L, s1, c1, s2, h)
        b = (c0 + h * s2, L, s1, c1, s2, c2 - h)
        return [(d, p) for d0, pp_ in [(0, a), (h * s2, b)]
                for d, p in [(d0 + dd, p2) for dd, p2 in _split_colpat(
                    (pp_[0], pp_[1], pp_[2], pp_[3], pp_[4], pp_[5]), max_free)]]
    if c1 > 1:
        h = c1 // 2
        a = (c0, L, s1, h, 0, 1)
        b = (c0 + h * s1, L, s1, c1 - h, 0, 1)
        out = []
        for base, pat in [(0, a), (h * s1, b)]:
            out.extend(_split_colpat(pat, max_free))
        return out
    h = L // 2
    a = (c0, h, 0, 1, 0, 1)
    b = (c0 + h, L - h, 0, 1, 0, 1)
    return _split_colpat(a, max_free) + _split_colpat(b, max_free)


def drain_split(stages_pp, max_free=288):
    out = []
    for (p, k, cmp_ops, cp_ops) in stages_pp:
        nc_ops = []
        for (r0, nr, dr, colpat, colB0) in cmp_ops:
            for (_, pat) in _split_colpat(colpat, max_free):
                nb0 = colB0 + (pat[0] - colpat[0])
                nc_ops.append((r0, nr, dr, pat, nb0))
        ncp_ops = []
        for (r0, nr, pat) in cp_ops:
            for (_, p2) in _split_colpat(pat, max_free):
                ncp_ops.append((r0, nr, p2))
        out.append((p, k, nc_ops, ncp_ops))
    return out


def gen_pingpong(n, nrows, ncols, p_min=1, max_free=288):
    global N, NROWS, NCOLS
    oldN, oldR, oldC = N, NROWS, NCOLS
    N, NROWS, NCOLS = n, nrows, ncols
    try:
        full = emit_pingpong()
        filt = [(p, k, c, cp) for (p, k, c, cp) in full if p >= p_min]
        return drain_split(filt, max_free)
    finally:
        N, NROWS, NCOLS = oldN, oldR, oldC


import bass_rust
import concourse.bacc as bacc
import concourse.mybir as mybir
from concourse import tile
from concourse.bass_utils import run_bass_kernel_spmd


B, D, H1, H2, F = 2048, 3072, 512, 256, 100
NCORES = 8
BS = B // NCORES            # 256 rows per core
LEAK = 0.2
P = 128
FL = 13                     # features per core (8*13 = 104 >= 100)
FPAD = NCORES * FL          # 104
NR, NC = 4, 512
RC = 2.0 ** 23              # rounding constant
QLEV = 8190.0
MRANGE = 16.0
QSCALE = QLEV / (2 * MRANGE)
DQ = (2 * MRANGE) / QLEV

f32 = mybir.dt.float32
f16 = mybir.dt.float16
bf16 = mybir.dt.bfloat16
i16 = mybir.dt.int16
i32 = mybir.dt.int32
AF = mybir.ActivationFunctionType
ALU = mybir.AluOpType

KD, K1, K2 = D // P, H1 // P, H2 // P     # 24, 4, 2
NCHUNK = 4                                 # DMA chunks for W1/x
KCH = KD // NCHUNK                         # 6 k-blocks per chunk


def sap(t_ap, pitch, pstart, pcount, coff, colpat):
    """Strided AP view: partitions [pstart, pstart+pcount), free pattern
    colpat=(c0,L,s1,c1,s2,c2) shifted to coff."""
    (c0, L, s1, c1, s2, c2) = colpat
    dims = [(pitch, pcount)]
    if c2 > 1:
        dims.append((s2, c2))
    if c1 > 1:
        dims.append((s1, c1))
    dims.append((1, L))
    a = t_ap.copy()
    a.ap = bass_rust.VecI64Pair(dims)
    a.offset = pstart * pitch + coff
    return a


SRC_OPS = gen_pingpong(256, 1, 256)
MRG_OPS = gen_pingpong(2048, 4, 512, p_min=256)


def emit_sort(nc, ops_table, rowpart, bufs, pitch,
              cp_engines, mir_pool=None):
    """Rotating 3-buffer odd-even merge sort (DVE min/max). The 3-buffer
    rotation avoids WAR stalls between consecutive stages. Cross-row compares
    read the B operand through an SBUF mirror copied by ACT/Pool
    (partition-shifted copies are legal in both directions)."""
    nb = len(bufs)
    ci = 0
    mi = 0
    for si, (p, k, cmp_ops, cp_ops) in enumerate(ops_table):
        cur = bufs[si % nb]
        nxt = bufs[(si + 1) % nb]
        for (r0, nr, dr, colpat, colB0) in cmp_ops:
            pa, pb = rowpart * r0, rowpart * (r0 + dr)
            npart = rowpart * nr
            a_in = sap(cur, pitch, pa, npart, colpat[0], colpat)
            a_out = sap(nxt, pitch, pa, npart, colpat[0], colpat)
            b_out = sap(nxt, pitch, pb, npart, colB0, colpat)
            if dr == 0:
                b_in = sap(cur, pitch, pb, npart, colB0, colpat)
            else:
                b_cur = sap(cur, pitch, pb, npart, colB0, colpat)
                mt = mir_pool.tile([128, 512], f32, tag="mir", bufs=4,
                                   name="mirt")
                b_in = sap(mt[:], mt[:].ap[0][0], pa, npart,
                           colpat[0], colpat)
                if mi % 2 == 0:
                    nc.scalar.copy(b_in, b_cur)
                else:
                    nc.gpsimd.tensor_copy(b_in, b_cur)
                mi += 1
            nc.vector.tensor_tensor(a_out, a_in, b_in, ALU.min)
            nc.vector.tensor_tensor(b_out, a_in, b_in, ALU.max)
        for (r0, nr, pat) in cp_ops:
            pa = rowpart * r0
            npart = rowpart * nr
            c_in = sap(cur, pitch, pa, npart, pat[0], pat)
            c_out = sap(nxt, pitch, pa, npart, pat[0], pat)
            cp_engines[ci % len(cp_engines)](c_out, c_in)
            ci += 1


def build_program():
    nc = bacc.Bacc(
        "TRN2", target_bir_lowering=False, debug=False, num_devices=NCORES)

    xTp = nc.dram_tensor("xTp", [P, KD * BS], bf16, kind="ExternalInput").ap()
    W1p = nc.dram_tensor("W1p", [P, KD * H1], bf16, kind="ExternalInput").ap()
    W2p = nc.dram_tensor("W2p", [P, K1 * H2], bf16, kind="ExternalInput").ap()
    Tp = nc.dram_tensor("Tp", [P, K2 * F], bf16, kind="ExternalInput").ap()
    Wfhp = nc.dram_tensor("Wfhp", [P, K2], bf16, kind="ExternalInput").ap()
    b1c = nc.dram_tensor("b1c", [P, K1], f32, kind="ExternalInput").ap()
    b2c = nc.dram_tensor("b2c", [P, K2], f32, kind="ExternalInput").ap()
    bfc = nc.dram_tensor("bfc", [1, 1], f32, kind="ExternalInput").ap()
    iotas = nc.dram_tensor("iotas", [P, BS], f32, kind="ExternalInput").ap()
    w16d = nc.dram_tensor("w16", [P, 1], f16, kind="ExternalInput").ap()
    pmaskd = nc.dram_tensor("pmask", [P, 1], i16, kind="ExternalInput").ap()
    lmA = nc.dram_tensor("lmA", [P, P], f32, kind="ExternalInput").ap()
    lmB = nc.dram_tensor("lmB", [P, P], f32, kind="ExternalInput").ap()
    out = nc.dram_tensor("out", [1, B], f32, kind="ExternalOutput").ap()
    outh = nc.dram_tensor("outh", [1, BS], f32, kind="ExternalOutput").ap()

    with tile.TileContext(nc) as tc:
        with (
            tc.tile_pool(name="persist", bufs=1) as pers,
            tc.tile_pool(name="dram", bufs=1, space="DRAM") as dpool,
        ):
            # ---- persistent tiles (loads deferred into the chunk stream) ----
            bf_sb = pers.tile([1, 1], f32)
            iota_sb = pers.tile([P, BS], f32)
            w16_sb = pers.tile([P, 1], f16)
            pmask_sb = pers.tile([P, 1], i16)
            lmA_sb = pers.tile([P, P], f32)
            lmB_sb = pers.tile([P, P], f32)
            b1_sb = pers.tile([P, K1], f32)
            b2_sb = pers.tile([P, K2], f32)
            bq_sb = pers.tile([P, 1], f32)
            nc.vector.memset(bq_sb[:], MRANGE * QSCALE)

            hWf_sb = pers.tile([1, BS], f32)

            # ======== phase 1: MLP (bf16 weights/activations) ========
            with (
                tc.tile_pool(name="mlp", bufs=1) as mp,
                tc.tile_pool(name="psum_mm", bufs=1, space="PSUM") as pmm,
            ):
                W2_sb = mp.tile([P, K1 * H2], bf16)
                T_sb = mp.tile([P, K2 * F], bf16)
                Wfh_sb = mp.tile([P, K2], bf16)

                xT_sb = mp.tile([P, KD * BS], bf16)
                W1_sb = mp.tile([P, KD * H1], bf16)
                for c in range(NCHUNK):
                    lo = c * KCH
                    nc.sync.dma_start(
                        W1_sb[:, lo * H1:(lo + KCH) * H1],
                        W1p[:, lo * H1:(lo + KCH) * H1])
                    nc.sync.dma_start(
                        xT_sb[:, lo * BS:(lo + KCH) * BS],
                        xTp[:, lo * BS:(lo + KCH) * BS])
                    if c == 0:
                        for dst, src in ((b1_sb, b1c), (b2_sb, b2c),
                                         (bf_sb, bfc), (iota_sb, iotas),
                                         (w16_sb, w16d), (pmask_sb, pmaskd),
                                         (lmA_sb, lmA), (lmB_sb, lmB)):
                            nc.sync.dma_start(dst[:], src)
                nc.sync.dma_start(W2_sb[:], W2p)
                nc.sync.dma_start(T_sb[:], Tp)
                nc.sync.dma_start(Wfh_sb[:], Wfhp)

                pt1 = [pmm.tile([P, BS], f32, name=f"pt1_{mb}")
                       for mb in range(K1)]
                for k in range(KD):
                    for mb in range(K1):
                        nc.tensor.matmul(
                            pt1[mb][:],
                            W1_sb[:, k * H1 + mb * P: k * H1 + (mb + 1) * P],
                            xT_sb[:, k * BS:(k + 1) * BS],
                            start=(k == 0), stop=(k == KD - 1))
                h1T = [mp.tile([P, BS], bf16, name=f"h1T{m}") for m in range(K1)]
                for mb in range(K1):
                    s1 = mp.tile([P, BS], f32, tag="stmp", bufs=2,
                                 name=f"s1_{mb}")
                    nc.scalar.activation(
                        s1[:], pt1[mb][:], AF.Identity, bias=b1_sb[:, mb:mb + 1])
                    nc.vector.scalar_tensor_tensor(
                        h1T[mb][:], s1[:], LEAK, s1[:], op0=ALU.mult,
                        op1=ALU.max)

                pt2 = [pmm.tile([P, BS], f32, name=f"pt2_{mb}")
                       for mb in range(K2)]
                for k in range(K1):
                    for mb in range(K2):
                        nc.tensor.matmul(
                            pt2[mb][:],
                            W2_sb[:, k * H2 + mb * P: k * H2 + (mb + 1) * P],
                            h1T[k][:],
                            start=(k == 0), stop=(k == K1 - 1))
                h2T = [mp.tile([P, BS], bf16, name=f"h2T{m}") for m in range(K2)]
                for mb in range(K2):
                    s2 = mp.tile([P, BS], f32, tag="stmp", bufs=2,
                                 name=f"s2_{mb}")
                    nc.scalar.activation(
                        s2[:], pt2[mb][:], AF.Identity, bias=b2_sb[:, mb:mb + 1])
                    nc.vector.scalar_tensor_tensor(
                        h2T[mb][:], s2[:], LEAK, s2[:], op0=ALU.mult,
                        op1=ALU.max)

                pt_m = pmm.tile([F, BS], f32, name="ptm")
                for k in range(K2):
                    nc.tensor.matmul(
                        pt_m[:], T_sb[:, k * F:(k + 1) * F], h2T[k][:],
                        start=(k == 0), stop=(k == K2 - 1))

                ph = pmm.tile([1, BS], f32, name="ph")
                for k in range(K2):
                    nc.tensor.matmul(
                        ph[:], Wfh_sb[:, k:k + 1], h2T[k][:],
                        start=(k == 0), stop=(k == K2 - 1))
                nc.vector.tensor_copy(hWf_sb[:], ph[:])

                # ---- quantize + pack straight from PSUM ----
                skey = pers.tile([P, BS], f32)
                nc.scalar.activation(
                    skey[:F, :], pt_m[:], AF.Identity, bias=bq_sb[:F, :],
                    scale=QSCALE)
            sktmp = pers.tile([P, BS], f32)
            sktmp2 = pers.tile([P, BS], f32)
            nc.vector.tensor_scalar(
                skey[:F, :], skey[:F, :], scalar1=RC, scalar2=RC,
                op0=ALU.add, op1=ALU.subtract)
            nc.gpsimd.tensor_scalar(
                skey[:F, :], skey[:F, :], scalar1=8191.0, scalar2=0.0,
                op0=ALU.min, op1=ALU.max)
            nc.vector.tensor_tensor(skey[:F, :], skey[:F, :], iota_sb[:F, :],
                                    ALU.add)
            spitch = skey[:].ap[0][0]
            emit_sort(nc, SRC_OPS, P, [skey[:], sktmp[:], sktmp2[:]], spitch,
                      [lambda o, i: nc.scalar.copy(o, i),
                       lambda o, i: nc.gpsimd.tensor_copy(o, i)])

            # ======== phase 3: AllToAll ========
            a2a_in = dpool.tile([FPAD, BS], f32)
            a2a_out = dpool.tile([FPAD, BS], f32)
            nc.sync.dma_start(a2a_in[:F, :], skey[:F, :])
            nc.sync.dma_start(a2a_in[F:FPAD, :], skey[:FPAD - F, :])
            nc.gpsimd.collective_compute(
                "AllToAll", ALU.bypass,
                replica_groups=[list(range(NCORES))],
                ins=[a2a_in.opt()], outs=[a2a_out.opt()])

            key = pers.tile([P, NC], f32)
            nc.vector.memset(key[:], 0.0)
            for e in range(NCORES):
                r, half = e // 2, e % 2
                nc.sync.dma_start(
                    key[32 * r:32 * r + FL, half * BS:(half + 1) * BS],
                    a2a_out[e * FL:(e + 1) * FL, :])

            # ======== phase 4: merge (30 stages) + scan + unsort ========
            pitch = key[:].ap[0][0]
            with (
                tc.tile_pool(name="sortp", bufs=1) as sp,
                tc.tile_pool(name="psum2", bufs=1, space="PSUM") as pp2,
            ):
                tmp = sp.tile([P, NC], f32)
                tmp2 = sp.tile([P, NC], f32)
                emit_sort(nc, MRG_OPS, 32, [key[:], tmp[:], tmp2[:]], pitch,
                          cp_engines=[lambda o, i: nc.scalar.copy(o, i),
                                      lambda o, i: nc.gpsimd.tensor_copy(o, i)],
                          mir_pool=sp)

                # ---- scan phase: split key = g + j/2048 ----
                kq = sp.tile([P, NC], f32)
                nc.vector.tensor_scalar_mul(kq[:], key[:], 2048.0)
                ki = sp.tile([P, NC], i32)
                nc.vector.tensor_copy(ki[:], kq[:])
                ji = sp.tile([P, NC], i32)
                nc.vector.tensor_scalar(
                    ji[:], ki[:], scalar1=2047, scalar2=None,
                    op0=ALU.bitwise_and)
                ji16 = sp.tile([P, NC], i16)
                nc.gpsimd.tensor_copy(ji16[:], ji[:])
                # ---- unsort index prep (overlaps ACT exps below) ----
                tt = sp.tile([P, NC], i16)
                nc.vector.tensor_scalar(
                    tt[:], ji16[:], scalar1=pmask_sb[:], scalar2=None,
                    op0=ALU.bitwise_or)
                neg1 = sp.tile([P, NC], i16)
                nc.gpsimd.memset(neg1[:], -1)
                m0 = sp.tile([P, NC], i16)
                nc.vector.tensor_scalar(
                    m0[:], tt[:], scalar1=1023, scalar2=None, op0=ALU.is_le)
                idx0 = sp.tile([P, NC], i16)
                nc.vector.select(idx0[:], m0[:], tt[:], neg1[:])
                t1 = sp.tile([P, NC], i16)
                nc.gpsimd.tensor_scalar(
                    t1[:], tt[:], scalar1=1024, scalar2=None, op0=ALU.subtract)
                idx1 = sp.tile([P, NC], i16)
                nc.vector.select(idx1[:], m0[:], neg1[:], t1[:])
                gi = sp.tile([P, NC], i32)
                nc.vector.tensor_scalar(
                    gi[:], ki[:], scalar1=-2048, scalar2=None,
                    op0=ALU.bitwise_and)
                g2k = sp.tile([P, NC], f32)
                nc.vector.tensor_copy(g2k[:], gi[:])
                bneg = sp.tile([P, 1], f32)
                nc.vector.memset(bneg[:], -MRANGE)
                bpos = sp.tile([P, 1], f32)
                nc.vector.memset(bpos[:], MRANGE)
                u = sp.tile([P, NC], f32)
                nc.scalar.activation(
                    u[:], g2k[:], AF.Exp, bias=bneg[:], scale=DQ / 2048.0)
                v = sp.tile([P, NC], f32)
                nc.scalar.activation(
                    v[:], g2k[:], AF.Exp, bias=bpos[:], scale=-DQ / 2048.0)

                su = sp.tile([P, NC], f32)
                nc.vector.tensor_tensor_scan(
                    su[:], u[:], u[:], initial=0.0, op0=ALU.add,
                    op1=ALU.bypass)
                sv = sp.tile([P, NC], f32)
                nc.vector.tensor_tensor_scan(
                    sv[:, NC - 1::-1], v[:, NC - 1::-1], v[:, NC - 1::-1],
                    initial=0.0, op0=ALU.add, op1=ALU.bypass)

                # cross-quadrant carries via masked prefix matmuls
                pcu = pp2.tile([P, 2], f32, name="pcu")
                nc.tensor.matmul(pcu[:, 0:1], lmA_sb[:], su[:, NC - 1:NC],
                                 start=True, stop=True)
                nc.tensor.matmul(pcu[:, 1:2], lmB_sb[:], sv[:, 0:1],
                                 start=True, stop=True)
                carr = sp.tile([P, 2], f32)
                nc.vector.tensor_copy(carr[:], pcu[:])

                s1u = sp.tile([P, NC], f32)
                nc.gpsimd.tensor_scalar(
                    s1u[:], su[:], scalar1=carr[:, 0:1], scalar2=None,
                    op0=ALU.add)
                s2vi = sp.tile([P, NC], f32)
                nc.vector.scalar_tensor_tensor(
                    s2vi[:], sv[:], carr[:, 1:2], v[:],
                    op0=ALU.add, op1=ALU.subtract)

                fa = sp.tile([P, NC], f32)
                nc.vector.tensor_tensor(fa[:], v[:], s1u[:], ALU.mult)
                fb = sp.tile([P, NC], f32)
                nc.gpsimd.tensor_tensor(fb[:], u[:], s2vi[:], ALU.mult)
                feats16 = sp.tile([P, NC], f16)
                nc.vector.tensor_tensor(feats16[:], fa[:], fb[:], ALU.add)

                # ---- unsort via local_scatter (j < 1024 | j >= 1024) ----
                dst0 = sp.tile([P, 2 * NC], f16)
                dst1 = sp.tile([P, 2 * NC], f16)
                nc.gpsimd.local_scatter(
                    dst0[:], feats16[:], idx0[:], channels=P,
                    num_elems=2 * NC, num_idxs=NC)
                nc.gpsimd.local_scatter(
                    dst1[:], feats16[:], idx1[:], channels=P,
                    num_elems=2 * NC, num_idxs=NC)

                contrib = sp.tile([1, B], f32)
                for h, dst in ((0, dst0), (1, dst1)):
                    for s in range(2):
                        pc = pp2.tile([1, NC], f32, tag="pc", bufs=1,
                                      name=f"pc{h}{s}")
                        nc.tensor.matmul(
                            pc[:], w16_sb[:], dst[:, s * NC:(s + 1) * NC],
                            start=True, stop=True)
                        nc.vector.tensor_copy(
                            contrib[:, h * 1024 + s * NC:
                                    h * 1024 + (s + 1) * NC], pc[:])
                nc.sync.dma_start(out[:], contrib[:])
                osb = sp.tile([1, BS], f32)
                nc.vector.tensor_scalar(
                    osb[:], hWf_sb[:], scalar1=bf_sb[:1, :1], scalar2=None,
                    op0=ALU.add)
                nc.sync.dma_start(outh[:], osb[:])

    nc.compile()
    return nc


def _build_in_maps(inputs):
    x = np.asarray(inputs["x"], np.float32)
    W1 = np.asarray(inputs["W1"], np.float32)
    b1 = np.asarray(inputs["b1"], np.float32)
    W2 = np.asarray(inputs["W2"], np.float32)
    b2 = np.asarray(inputs["b2"], np.float32)
    T = np.asarray(inputs["T"], np.float32)
    Wf = np.asarray(inputs["Wf"], np.float32)
    bf = np.asarray(inputs["bf"], np.float32)

    bfl = ml_dtypes.bfloat16
    W1p = np.ascontiguousarray(
        W1.reshape(KD, P, H1).transpose(1, 0, 2).reshape(P, KD * H1)
    ).astype(bfl)
    W2p = np.ascontiguousarray(
        W2.reshape(K1, P, H2).transpose(1, 0, 2).reshape(P, K1 * H2)
    ).astype(bfl)
    Tp = np.ascontiguousarray(
        T.reshape(K2, P, F).transpose(1, 0, 2).reshape(P, K2 * F)
    ).astype(bfl)
    Wfhp = np.ascontiguousarray(
        Wf[:H2].reshape(K2, P).T).astype(bfl)
    b1p = np.ascontiguousarray(b1.reshape(K1, P).T)
    b2p = np.ascontiguousarray(b2.reshape(K2, P).T)

    wff = Wf[H2:, 0]
    wff_pad = np.zeros(FPAD, np.float32)
    wff_pad[:F] = wff

    lmaskA = np.zeros((P, P), np.float32)
    lmaskB = np.zeros((P, P), np.float32)
    for k in range(P):
        for m in range(P):
            if k % 32 == m % 32:
                if k // 32 < m // 32:
                    lmaskA[k, m] = 1.0
                elif k // 32 > m // 32:
                    lmaskB[k, m] = 1.0

    pmask = np.full((P, 1), -1, np.int16)
    for r in range(NR):
        pmask[32 * r:32 * r + FL] = 0

    common = {
        "W1p": W1p, "W2p": W2p, "Tp": Tp, "Wfhp": Wfhp,
        "b1c": b1p, "b2c": b2p,
        "bfc": np.ascontiguousarray(bf.reshape(1, 1)),
        "lmA": lmaskA, "lmB": lmaskB, "pmask": pmask,
    }
    in_maps = []
    for d in range(NCORES):
        m = dict(common)
        xT = x[d * BS:(d + 1) * BS, :].T
        m["xTp"] = np.ascontiguousarray(
            xT.reshape(KD, P, BS).transpose(1, 0, 2).reshape(P, KD * BS)
        ).astype(bfl)
        w16 = np.zeros((P, 1), np.float16)
        for r in range(NR):
            w16[32 * r:32 * r + FL, 0] = wff_pad[d * FL:(d + 1) * FL]
        m["w16"] = w16
        iot = np.broadcast_to(
            (d * BS + np.arange(BS, dtype=np.float32)) / 2048.0, (P, BS))
        m["iotas"] = np.ascontiguousarray(iot.astype(np.float32))
        in_maps.append(m)
    return in_maps


_NC_CACHE = None


def _get_program():
    global _NC_CACHE
    if _NC_CACHE is None:
        _NC_CACHE = build_program()
    return _NC_CACHE


def kernel(x, W1, b1, W2, b2, T, Wf, bf):
    nc = _get_program()
    in_maps = _build_in_maps(dict(
        x=x, W1=W1, b1=b1, W2=W2, b2=b2, T=T, Wf=Wf, bf=bf))
    res = run_bass_kernel_spmd(nc, in_maps, core_ids=list(range(NCORES)))
    total = np.zeros(B, np.float64)
    for d in range(NCORES):
        total += res.results[d]["out"].ravel().astype(np.float64)
        total[d * BS:(d + 1) * BS] += res.results[d]["outh"].ravel()
    return total.reshape(B, 1).astype(np.float32)


# revision 25
# speedup vs baseline: 2.0202x; 1.0375x over previous
"""Trainium2 Bass kernel for nn_Discriminator: MLP + sort-based minibatch
discrimination with gpsimd local_scatter un-permutation. Self-contained."""
import numpy as np
import ml_dtypes

N = 2048
NROWS = 4
NCOLS = 512


def stages(n=None):
    if n is None:
        n = N
    out = []
    p = 1
    while p < n:
        k = p
        while k >= 1:
            lefts = []
            j = k % p
            while j <= n - 1 - k:
                for i in range(0, min(k, n - j - k)):
                    x = i + j
                    if (x // (2 * p)) == ((x + k) // (2 * p)):
                        lefts.append(x)
                j += 2 * k
            out.append((p, k, np.array(sorted(lefts), dtype=np.int64)))
            k //= 2
        p *= 2
    return out


def runs_of(xs):
    """Compress sorted ints into <=3-level pattern (start, L, s1, c1, s2, c2)."""
    xs = np.asarray(xs)
    if len(xs) == 0:
        return None
    breaks = np.where(np.diff(xs) != 1)[0]
    starts_i = np.concatenate([[0], breaks + 1])
    ends_i = np.concatenate([breaks, [len(xs) - 1]])
    run_starts = xs[starts_i]
    run_lens = ends_i - starts_i + 1
    if not np.all(run_lens == run_lens[0]):
        return None
    L = int(run_lens[0])
    if len(run_starts) == 1:
        return (int(run_starts[0]), L, 0, 1, 0, 1)
    d = np.diff(run_starts)
    if np.all(d == d[0]):
        return (int(run_starts[0]), L, int(d[0]), len(run_starts), 0, 1)
    s1 = d[0]
    c1 = 1
    while c1 < len(d) and d[c1 - 1] == s1:
        c1 += 1
    group = c1
    if len(run_starts) % group != 0:
        return None
    rs = run_starts.reshape(-1, group)
    inner = np.diff(rs, axis=1)
    starts2 = rs[:, 0]
    d2 = np.diff(starts2)
    if inner.size and not np.all(inner == s1):
        return None
    if len(d2) and not np.all(d2 == d2[0]):
        return None
    return (int(run_starts[0]), L, int(s1), group,
            int(d2[0]) if len(d2) else 0, len(starts2))


def emit_ops():
    """Returns list of (p, k, [ops]); op = (r0, nrows, drow, colpat, colB0)."""
    all_stages = []
    for (p, k, lefts) in stages():
        ops = []
        rows = lefts // NCOLS
        cols = lefts % NCOLS
        drows = (lefts + k) // NCOLS - rows
        for dr in np.unique(drows):
            sel = drows == dr
            rset = np.unique(rows[sel])
            cset = np.unique(cols[sel])
            assert sel.sum() == len(rset) * len(cset), (p, k, dr)
            for r in rset:
                cc = np.sort(cols[sel & (rows == r)])
                assert np.array_equal(cc, cset), (p, k, dr, r)
            colpat = runs_of(cset)
            assert colpat is not None, (p, k, dr, cset[:20])
            rpat = runs_of(rset)
            assert rpat is not None, (p, k, dr, rset)
            (r0, Lr, sr1, cr1, sr2, cr2) = rpat
            assert sr2 == 0 and cr2 == 1, (p, k, dr, rpat)
            colB0 = int((cset[0] + k) % NCOLS)
            for g in range(cr1):
                rstart = r0 + g * sr1
                ops.append((int(rstart), int(Lr), int(dr), colpat, colB0))
        all_stages.append((p, int(k), ops))
    return all_stages


def _row_chunks(a_base, b_base, nr):
    allowed = {0: 4, 1: 1, 2: 2, 3: 1}
    out = []
    off = 0
    while off < nr:
        c = min(allowed[(a_base + off) % 4], allowed[(b_base + off) % 4], nr - off)
        out.append((off, c))
        off += c
    return out


def legalize(all_stages):
    out = []
    for (p, k, ops) in all_stages:
        nops = []
        for (r0, nr, dr, colpat, colB0) in ops:
            for (off, c) in _row_chunks(r0, r0 + dr, nr):
                nops.append((r0 + off, c, dr, colpat, colB0))
        out.append((p, k, nops))
    return out


def colpat_idx(colpat):
    (c0, L, s1, c1, s2, c2) = colpat
    return (c0 + np.arange(c2)[:, None, None] * s2
            + np.arange(c1)[None, :, None] * s1
            + np.arange(L)[None, None, :]).ravel()


def runs_multi(xs, max_groups=6):
    xs = np.asarray(xs)
    if len(xs) == 0:
        return []
    r = runs_of(xs)
    if r is not None:
        return [r]
    breaks = np.where(np.diff(xs) != 1)[0]
    starts_i = np.concatenate([[0], breaks + 1])
    ends_i = np.concatenate([breaks, [len(xs) - 1]])
    run_starts = xs[starts_i]
    run_lens = ends_i - starts_i + 1
    out = []
    for L in np.unique(run_lens):
        sel = run_lens == L
        rs = run_starts[sel]
        d = np.diff(rs)
        if len(d) == 0 or np.all(d == d[0]):
            out.append((int(rs[0]), int(L), int(d[0]) if len(d) else 0,
                        len(rs), 0, 1))
        else:
            for s in rs:
                out.append((int(s), int(L), 0, 1, 0, 1))
    return out


def emit_pingpong():
    """cp op = (r0, nr, pat, old): old=True -> the cell was untouched in the
    previous stage too, so it can be copied from the 2-stages-old rotation
    buffer (dependency jumps a stage back; copy leaves the critical chain)."""
    out = []
    prev_touched = np.ones((NROWS, NCOLS), dtype=bool)
    for (p, k, ops) in legalize(emit_ops()):
        touched = np.zeros((NROWS, NCOLS), dtype=bool)
        for (r0, nr, dr, colpat, colB0) in ops:
            ia = colpat_idx(colpat)
            ib = ia + (colB0 - colpat[0])
            for rr in range(r0, r0 + nr):
                touched[rr, ia] = True
                touched[rr + dr, ib] = True
        cp_ops = []
        for old in (False, True):
            need = (~touched) & (prev_touched if not old else ~prev_touched)
            r = 0
            while r < NROWS:
                mask = need[r]
                r2 = r + 1
                while r2 < NROWS and np.array_equal(need[r2], mask):
                    r2 += 1
                cols = np.where(mask)[0]
                if len(cols):
                    for pat in runs_multi(cols):
                        off = 0
                        nr_ = r2 - r
                        allowed = {0: 4, 1: 1, 2: 2, 3: 1}
                        while off < nr_:
                            c = min(allowed[(r + off) % 4], nr_ - off)
                            cp_ops.append((r + off, c, pat, old))
                            off += c
                r = r2
        prev_touched = touched
        out.append((p, k, ops, cp_ops))
    return out


def _split_colpat(colpat, max_free=288):
    (c0, L, s1, c1, s2, c2) = colpat
    free = L * c1 * c2
    if free <= max_free:
        return [(0, colpat)]
    if c2 > 1:
        h = c2 // 2
        a = (c0, L, s1, c1, s2, h)
        b = (c0 + h * s2, L, s1, c1, s2, c2 - h)
        return [(d, p) for d0, pp_ in [(0, a), (h * s2, b)]
                for d, p in [(d0 + dd, p2) for dd, p2 in _split_colpat(
                    (pp_[0], pp_[1], pp_[2], pp_[3], pp_[4], pp_[5]), max_free)]]
    if c1 > 1:
        h = c1 // 2
        a = (c0, L, s1, h, 0, 1)
        b = (c0 + h * s1, L, s1, c1 - h, 0, 1)
        out = []
        for base, pat in [(0, a), (h * s1, b)]:
            out.extend(_split_colpat(pat, max_free))
        return out
    h = L // 2
    a = (c0, h, 0, 1, 0, 1)
    b = (c0 + h, L - h, 0, 1, 0, 1)
    return _split_colpat(a, max_free) + _split_colpat(b, max_free)


def drain_split(stages_pp, max_free=288):
    out = []
    for (p, k, cmp_ops, cp_ops) in stages_pp:
        nc_ops = []
        for (r0, nr, dr, colpat, colB0) in cmp_ops:
            for (_, pat) in _split_colpat(colpat, max_free):
                nb0 = colB0 + (pat[0] - colpat[0])
                nc_ops.append((r0, nr, dr, pat, nb0))
        ncp_ops = []
        for (r0, nr, pat, old) in cp_ops:
            for (_, p2) in _split_colpat(pat, max_free):
                ncp_ops.append((r0, nr, p2, old))
        out.append((p, k, nc_ops, ncp_ops))
    return out


def gen_pingpong(n, nrows, ncols, p_min=1, max_free=288):
    global N, NROWS, NCOLS
    oldN, oldR, oldC = N, NROWS, NCOLS
    N, NROWS, NCOLS = n, nrows, ncols
    try:
        full = emit_pingpong()
        filt = [(p, k, c, cp) for (p, k, c, cp) in full if p >= p_min]
        return drain_split(filt, max_free)
    finally:
        N, NROWS, NCOLS = oldN, oldR, oldC


import bass_rust
import concourse.bacc as bacc
import concourse.mybir as mybir
from concourse import tile
from concourse.bass_utils import run_bass_kernel_spmd


B, D, H1, H2, F = 2048, 3072, 512, 256, 100
NCORES = 8
BS = B // NCORES            # 256 rows per core
LEAK = 0.2
P = 128
FL = 13                     # features per core (8*13 = 104 >= 100)
FPAD = NCORES * FL          # 104
NR, NC = 4, 512
RC = 2.0 ** 23              # rounding constant
QLEV = 8190.0
MRANGE = 16.0
QSCALE = QLEV / (2 * MRANGE)
DQ = (2 * MRANGE) / QLEV

f32 = mybir.dt.float32
f16 = mybir.dt.float16
bf16 = mybir.dt.bfloat16
i16 = mybir.dt.int16
i32 = mybir.dt.int32
AF = mybir.ActivationFunctionType
ALU = mybir.AluOpType

KD, K1, K2 = D // P, H1 // P, H2 // P     # 24, 4, 2
NCHUNK = 4                                 # DMA chunks for W1/x
KCH = KD // NCHUNK                         # 6 k-blocks per chunk


def sap(t_ap, pitch, pstart, pcount, coff, colpat):
    """Strided AP view: partitions [pstart, pstart+pcount), free pattern
    colpat=(c0,L,s1,c1,s2,c2) shifted to coff."""
    (c0, L, s1, c1, s2, c2) = colpat
    dims = [(pitch, pcount)]
    if c2 > 1:
        dims.append((s2, c2))
    if c1 > 1:
        dims.append((s1, c1))
    dims.append((1, L))
    a = t_ap.copy()
    a.ap = bass_rust.VecI64Pair(dims)
    a.offset = pstart * pitch + coff
    return a


SRC_OPS = gen_pingpong(256, 1, 256)
MRG_OPS = gen_pingpong(2048, 4, 512, p_min=256)


def emit_sort(nc, ops_table, rowpart, bufs, pitch,
              cp_engines, mir_pool=None):
    """Rotating 3-buffer odd-even merge sort (DVE min/max). The 3-buffer
    rotation avoids WAR stalls between consecutive stages. Cross-row compares
    read the B operand through an SBUF mirror copied by ACT/Pool
    (partition-shifted copies are legal in both directions)."""
    nb = len(bufs)
    ci = 0
    mi = 0
    for si, (p, k, cmp_ops, cp_ops) in enumerate(ops_table):
        cur = bufs[si % nb]
        nxt = bufs[(si + 1) % nb]
        for (r0, nr, dr, colpat, colB0) in cmp_ops:
            pa, pb = rowpart * r0, rowpart * (r0 + dr)
            npart = rowpart * nr
            a_in = sap(cur, pitch, pa, npart, colpat[0], colpat)
            a_out = sap(nxt, pitch, pa, npart, colpat[0], colpat)
            b_out = sap(nxt, pitch, pb, npart, colB0, colpat)
            if dr == 0:
                b_in = sap(cur, pitch, pb, npart, colB0, colpat)
            else:
                b_cur = sap(cur, pitch, pb, npart, colB0, colpat)
                mt = mir_pool.tile([128, 512], f32, tag="mir", bufs=4,
                                   name="mirt")
                b_in = sap(mt[:], mt[:].ap[0][0], pa, npart,
                           colpat[0], colpat)
                if mi % 2 == 0:
                    nc.scalar.copy(b_in, b_cur)
                else:
                    nc.gpsimd.tensor_copy(b_in, b_cur)
                mi += 1
            nc.vector.tensor_tensor(a_out, a_in, b_in, ALU.min)
            nc.vector.tensor_tensor(b_out, a_in, b_in, ALU.max)
        for (r0, nr, pat, old) in cp_ops:
            pa = rowpart * r0
            npart = rowpart * nr
            src = bufs[(si - 1) % nb] if (old and si > 0) else cur
            c_in = sap(src, pitch, pa, npart, pat[0], pat)
            c_out = sap(nxt, pitch, pa, npart, pat[0], pat)
            cp_engines[ci % len(cp_engines)](c_out, c_in)
            ci += 1


def build_program():
    nc = bacc.Bacc(
        "TRN2", target_bir_lowering=False, debug=False, num_devices=NCORES)

    SM = 521                       # packed smalls: iota|lmA|lmB|b1|b2|w16|pmask|bf
    WPK = K1 * H2 + K2 * F + K2    # packed W2|T|Wfh

    xTp = nc.dram_tensor("xTp", [P, KD * BS], bf16, kind="ExternalInput").ap()
    W1p = nc.dram_tensor("W1p", [P, KD * H1], bf16, kind="ExternalInput").ap()
    wpkd = nc.dram_tensor("wpk", [P, WPK], bf16, kind="ExternalInput").ap()
    smd = nc.dram_tensor("sm", [P, SM], f32, kind="ExternalInput").ap()
    outc = nc.dram_tensor("outc", [1, B + BS], f32, kind="ExternalOutput").ap()

    with tile.TileContext(nc) as tc:
        with (
            tc.tile_pool(name="persist", bufs=1) as pers,
            tc.tile_pool(name="dram", bufs=1, space="DRAM") as dpool,
        ):
            # ---- packed persistent tile + views ----
            sm_sb = pers.tile([P, SM], f32)
            w16_v = sm_sb[:].bitcast(f16)[:, 2 * 518:2 * 518 + 1]
            pmask_v = sm_sb[:].bitcast(i16)[:, 2 * 519:2 * 519 + 1]
            bq_sb = pers.tile([P, 1], f32)
            nc.vector.memset(bq_sb[:], MRANGE * QSCALE)

            hWf_sb = pers.tile([1, BS], f32)

            # ======== phase 1: MLP (bf16 weights/activations) ========
            with (
                tc.tile_pool(name="mlp", bufs=1) as mp,
                tc.tile_pool(name="psum_mm", bufs=1, space="PSUM") as pmm,
            ):
                wpk_sb = mp.tile([P, WPK], bf16)

                xT_sb = mp.tile([P, KD * BS], bf16)
                W1_sb = mp.tile([P, KD * H1], bf16)
                for c in range(NCHUNK):
                    lo = c * KCH
                    nc.sync.dma_start(
                        W1_sb[:, lo * H1:(lo + KCH) * H1],
                        W1p[:, lo * H1:(lo + KCH) * H1])
                    nc.sync.dma_start(
                        xT_sb[:, lo * BS:(lo + KCH) * BS],
                        xTp[:, lo * BS:(lo + KCH) * BS])
                    if c == 0:
                        nc.sync.dma_start(sm_sb[:], smd)
                        nc.sync.dma_start(wpk_sb[:], wpkd)

                pt1 = [pmm.tile([P, BS], f32, name=f"pt1_{mb}")
                       for mb in range(K1)]
                for k in range(KD):
                    for mb in range(K1):
                        nc.tensor.matmul(
                            pt1[mb][:],
                            W1_sb[:, k * H1 + mb * P: k * H1 + (mb + 1) * P],
                            xT_sb[:, k * BS:(k + 1) * BS],
                            start=(k == 0), stop=(k == KD - 1))
                h1T = [mp.tile([P, BS], bf16, name=f"h1T{m}") for m in range(K1)]
                for mb in range(K1):
                    s1 = mp.tile([P, BS], f32, tag="stmp", bufs=2,
                                 name=f"s1_{mb}")
                    nc.scalar.activation(
                        s1[:], pt1[mb][:], AF.Identity,
                        bias=sm_sb[:, 512 + mb:513 + mb])
                    nc.vector.scalar_tensor_tensor(
                        h1T[mb][:], s1[:], LEAK, s1[:], op0=ALU.mult,
                        op1=ALU.max)

                pt2 = [pmm.tile([P, BS], f32, name=f"pt2_{mb}")
                       for mb in range(K2)]
                for k in range(K1):
                    for mb in range(K2):
                        nc.tensor.matmul(
                            pt2[mb][:],
                            wpk_sb[:, k * H2 + mb * P: k * H2 + (mb + 1) * P],
                            h1T[k][:],
                            start=(k == 0), stop=(k == K1 - 1))
                h2T = [mp.tile([P, BS], bf16, name=f"h2T{m}") for m in range(K2)]
                for mb in range(K2):
                    s2 = mp.tile([P, BS], f32, tag="stmp", bufs=2,
                                 name=f"s2_{mb}")
                    nc.scalar.activation(
                        s2[:], pt2[mb][:], AF.Identity,
                        bias=sm_sb[:, 516 + mb:517 + mb])
                    nc.vector.scalar_tensor_tensor(
                        h2T[mb][:], s2[:], LEAK, s2[:], op0=ALU.mult,
                        op1=ALU.max)

                pt_m = pmm.tile([F, BS], f32, name="ptm")
                for k in range(K2):
                    nc.tensor.matmul(
                        pt_m[:], wpk_sb[:, K1 * H2 + k * F:K1 * H2 + (k + 1) * F],
                        h2T[k][:],
                        start=(k == 0), stop=(k == K2 - 1))

                ph = pmm.tile([1, BS], f32, name="ph")
                for k in range(K2):
                    nc.tensor.matmul(
                        ph[:], wpk_sb[:, K1 * H2 + K2 * F + k:K1 * H2 + K2 * F + k + 1],
                        h2T[k][:],
                        start=(k == 0), stop=(k == K2 - 1))
                nc.vector.tensor_copy(hWf_sb[:], ph[:])

                # ---- quantize + pack straight from PSUM ----
                skey = pers.tile([P, BS], f32)
                nc.scalar.activation(
                    skey[:F, :], pt_m[:], AF.Identity, bias=bq_sb[:F, :],
                    scale=QSCALE)
            sktmp = pers.tile([P, BS], f32)
            sktmp2 = pers.tile([P, BS], f32)
            nc.vector.tensor_scalar(
                skey[:F, :], skey[:F, :], scalar1=RC, scalar2=RC,
                op0=ALU.add, op1=ALU.subtract)
            nc.gpsimd.tensor_scalar(
                skey[:F, :], skey[:F, :], scalar1=8191.0, scalar2=0.0,
                op0=ALU.min, op1=ALU.max)
            nc.vector.tensor_tensor(skey[:F, :], skey[:F, :],
                                    sm_sb[:F, 0:BS], ALU.add)
            spitch = skey[:].ap[0][0]
            emit_sort(nc, SRC_OPS, P, [skey[:], sktmp[:], sktmp2[:]], spitch,
                      [lambda o, i: nc.scalar.copy(o, i),
                       lambda o, i: nc.gpsimd.tensor_copy(o, i)])

            # ======== phase 3: AllToAll ========
            a2a_in = dpool.tile([FPAD, BS], f32)
            a2a_out = dpool.tile([FPAD, BS], f32)
            nc.sync.dma_start(a2a_in[:F, :], skey[:F, :])
            nc.sync.dma_start(a2a_in[F:FPAD, :], skey[:FPAD - F, :])
            nc.gpsimd.collective_compute(
                "AllToAll", ALU.bypass,
                replica_groups=[list(range(NCORES))],
                ins=[a2a_in.opt()], outs=[a2a_out.opt()])

            key = pers.tile([P, NC], f32)
            nc.vector.memset(key[:], 0.0)
            # fancy-AP DMAs (one per quadrant row): a2a_out rows (2r+h)*13+f,
            # col i -> key[32r+f, h*256+i]
            kpitch = key[:].ap[0][0]
            for r in range(4):
                kdst = key[:].copy()
                kdst.ap = bass_rust.VecI64Pair([(kpitch, FL), (1, 2 * BS)])
                kdst.offset = (32 * r) * kpitch
                ksrc = a2a_out[:, :].copy()
                ksrc.ap = bass_rust.VecI64Pair(
                    [(BS, FL), (FL * BS, 2), (1, BS)])
                ksrc.offset = r * 2 * FL * BS
                nc.sync.dma_start(kdst, ksrc)

            # ======== phase 4: merge (30 stages) + scan + unsort ========
            pitch = key[:].ap[0][0]
            with (
                tc.tile_pool(name="sortp", bufs=1) as sp,
                tc.tile_pool(name="psum2", bufs=1, space="PSUM") as pp2,
            ):
                tmp = sp.tile([P, NC], f32)
                tmp2 = sp.tile([P, NC], f32)
                emit_sort(nc, MRG_OPS, 32, [key[:], tmp[:], tmp2[:]], pitch,
                          cp_engines=[lambda o, i: nc.scalar.copy(o, i),
                                      lambda o, i: nc.gpsimd.tensor_copy(o, i)],
                          mir_pool=sp)

                # ---- scan phase: split key = g + j/2048 ----
                kq = sp.tile([P, NC], f32)
                nc.vector.tensor_scalar_mul(kq[:], key[:], 2048.0)
                ki = sp.tile([P, NC], i32)
                nc.vector.tensor_copy(ki[:], kq[:])
                ji = sp.tile([P, NC], i32)
                nc.vector.tensor_scalar(
                    ji[:], ki[:], scalar1=2047, scalar2=None,
                    op0=ALU.bitwise_and)
                ji16 = sp.tile([P, NC], i16)
                nc.gpsimd.tensor_copy(ji16[:], ji[:])
                # ---- unsort index prep (overlaps ACT exps below) ----
                tt = sp.tile([P, NC], i16)
                nc.vector.tensor_scalar(
                    tt[:], ji16[:], scalar1=pmask_v, scalar2=None,
                    op0=ALU.bitwise_or)
                neg1 = sp.tile([P, NC], i16)
                nc.gpsimd.memset(neg1[:], -1)
                m0 = sp.tile([P, NC], i16)
                nc.vector.tensor_scalar(
                    m0[:], tt[:], scalar1=1023, scalar2=None, op0=ALU.is_le)
                idx0 = sp.tile([P, NC], i16)
                nc.vector.select(idx0[:], m0[:], tt[:], neg1[:])
                t1 = sp.tile([P, NC], i16)
                nc.gpsimd.tensor_scalar(
                    t1[:], tt[:], scalar1=1024, scalar2=None, op0=ALU.subtract)
                idx1 = sp.tile([P, NC], i16)
                nc.vector.select(idx1[:], m0[:], neg1[:], t1[:])
                gi = sp.tile([P, NC], i32)
                nc.vector.tensor_scalar(
                    gi[:], ki[:], scalar1=-2048, scalar2=None,
                    op0=ALU.bitwise_and)
                g2k = sp.tile([P, NC], f32)
                nc.vector.tensor_copy(g2k[:], gi[:])
                bneg = sp.tile([P, 1], f32)
                nc.vector.memset(bneg[:], -MRANGE)
                bpos = sp.tile([P, 1], f32)
                nc.vector.memset(bpos[:], MRANGE)
                u = sp.tile([P, NC], f32)
                nc.scalar.activation(
                    u[:], g2k[:], AF.Exp, bias=bneg[:], scale=DQ / 2048.0)
                v = sp.tile([P, NC], f32)
                nc.scalar.activation(
                    v[:], g2k[:], AF.Exp, bias=bpos[:], scale=-DQ / 2048.0)

                su = sp.tile([P, NC], f32)
                nc.vector.tensor_tensor_scan(
                    su[:], u[:], u[:], initial=0.0, op0=ALU.add,
                    op1=ALU.bypass)
                sv = sp.tile([P, NC], f32)
                nc.vector.tensor_tensor_scan(
                    sv[:, NC - 1::-1], v[:, NC - 1::-1], v[:, NC - 1::-1],
                    initial=0.0, op0=ALU.add, op1=ALU.bypass)

                # cross-quadrant carries via masked prefix matmuls
                pcu = pp2.tile([P, 2], f32, name="pcu")
                nc.tensor.matmul(pcu[:, 0:1], sm_sb[:, BS:BS + P], su[:, NC - 1:NC],
                                 start=True, stop=True)
                nc.tensor.matmul(pcu[:, 1:2], sm_sb[:, BS + P:BS + 2 * P], sv[:, 0:1],
                                 start=True, stop=True)
                carr = sp.tile([P, 2], f32)
                nc.vector.tensor_copy(carr[:], pcu[:])

                s1u = sp.tile([P, NC], f32)
                nc.gpsimd.tensor_scalar(
                    s1u[:], su[:], scalar1=carr[:, 0:1], scalar2=None,
                    op0=ALU.add)
                s2vi = sp.tile([P, NC], f32)
                nc.vector.scalar_tensor_tensor(
                    s2vi[:], sv[:], carr[:, 1:2], v[:],
                    op0=ALU.add, op1=ALU.subtract)

                fa = sp.tile([P, NC], f32)
                nc.vector.tensor_tensor(fa[:], v[:], s1u[:], ALU.mult)
                fb = sp.tile([P, NC], f32)
                nc.gpsimd.tensor_tensor(fb[:], u[:], s2vi[:], ALU.mult)
                feats16 = sp.tile([P, NC], f16)
                nc.vector.tensor_tensor(feats16[:], fa[:], fb[:], ALU.add)

                # ---- unsort via local_scatter (j < 1024 | j >= 1024) ----
                dst0 = sp.tile([P, 2 * NC], f16)
                dst1 = sp.tile([P, 2 * NC], f16)
                nc.gpsimd.local_scatter(
                    dst0[:], feats16[:], idx0[:], channels=P,
                    num_elems=2 * NC, num_idxs=NC)
                nc.gpsimd.local_scatter(
                    dst1[:], feats16[:], idx1[:], channels=P,
                    num_elems=2 * NC, num_idxs=NC)

                octile = sp.tile([1, B + BS], f32)
                nc.vector.tensor_scalar(
                    octile[:, B:B + BS], hWf_sb[:],
                    scalar1=sm_sb[0:1, 520:521], scalar2=None, op0=ALU.add)
                for h, dst in ((0, dst0), (1, dst1)):
                    for s in range(2):
                        pc = pp2.tile([1, NC], f32, tag="pc", bufs=1,
                                      name=f"pc{h}{s}")
                        nc.tensor.matmul(
                            pc[:], w16_v, dst[:, s * NC:(s + 1) * NC],
                            start=True, stop=True)
                        nc.vector.tensor_copy(
                            octile[:, h * 1024 + s * NC:
                                   h * 1024 + (s + 1) * NC], pc[:])
                nc.sync.dma_start(outc[:], octile[:])

    nc.compile()
    return nc


def _build_in_maps(inputs):
    x = np.asarray(inputs["x"], np.float32)
    W1 = np.asarray(inputs["W1"], np.float32)
    b1 = np.asarray(inputs["b1"], np.float32)
    W2 = np.asarray(inputs["W2"], np.float32)
    b2 = np.asarray(inputs["b2"], np.float32)
    T = np.asarray(inputs["T"], np.float32)
    Wf = np.asarray(inputs["Wf"], np.float32)
    bf = np.asarray(inputs["bf"], np.float32)

    bfl = ml_dtypes.bfloat16
    W1p = np.ascontiguousarray(
        W1.reshape(KD, P, H1).transpose(1, 0, 2).reshape(P, KD * H1)
    ).astype(bfl)
    W2p = W2.reshape(K1, P, H2).transpose(1, 0, 2).reshape(P, K1 * H2)
    Tp = T.reshape(K2, P, F).transpose(1, 0, 2).reshape(P, K2 * F)
    Wfhp = Wf[:H2].reshape(K2, P).T
    wpk = np.ascontiguousarray(
        np.concatenate([W2p, Tp, Wfhp], axis=1)).astype(bfl)
    b1p = b1.reshape(K1, P).T
    b2p = b2.reshape(K2, P).T

    wff = Wf[H2:, 0]
    wff_pad = np.zeros(FPAD, np.float32)
    wff_pad[:F] = wff

    lmaskA = np.zeros((P, P), np.float32)
    lmaskB = np.zeros((P, P), np.float32)
    for k in range(P):
        for m in range(P):
            if k % 32 == m % 32:
                if k // 32 < m // 32:
                    lmaskA[k, m] = 1.0
                elif k // 32 > m // 32:
                    lmaskB[k, m] = 1.0

    pmask = np.full(P, -1, np.int16)
    for r in range(NR):
        pmask[32 * r:32 * r + FL] = 0

    in_maps = []
    for d in range(NCORES):
        m = {"W1p": W1p, "wpk": wpk}
        xT = x[d * BS:(d + 1) * BS, :].T
        m["xTp"] = np.ascontiguousarray(
            xT.reshape(KD, P, BS).transpose(1, 0, 2).reshape(P, KD * BS)
        ).astype(bfl)
        w16 = np.zeros(P, np.float16)
        for r in range(NR):
            w16[32 * r:32 * r + FL] = wff_pad[d * FL:(d + 1) * FL]
        sm = np.zeros((P, 521), np.float32)
        sm[:, 0:BS] = (d * BS + np.arange(BS, dtype=np.float32)) / 2048.0
        sm[:, BS:BS + P] = lmaskA
        sm[:, BS + P:BS + 2 * P] = lmaskB
        sm[:, 512:512 + K1] = b1p
        sm[:, 516:516 + K2] = b2p
        smv = sm.view(np.uint16)
        smv[:, 2 * 518] = w16.view(np.uint16)
        smv[:, 2 * 519] = pmask.view(np.uint16)
        sm[0, 520] = bf[0]
        m["sm"] = sm
        in_maps.append(m)
    return in_maps


_NC_CACHE = None


def _get_program():
    global _NC_CACHE
    if _NC_CACHE is None:
        _NC_CACHE = build_program()
    return _NC_CACHE


def kernel(x, W1, b1, W2, b2, T, Wf, bf):
    nc = _get_program()
    in_maps = _build_in_maps(dict(
        x=x, W1=W1, b1=b1, W2=W2, b2=b2, T=T, Wf=Wf, bf=bf))
    res = run_bass_kernel_spmd(nc, in_maps, core_ids=list(range(NCORES)))
    total = np.zeros(B, np.float64)
    for d in range(NCORES):
        oc = res.results[d]["outc"].ravel()
        total += oc[:B].astype(np.float64)
        total[d * BS:(d + 1) * BS] += oc[B:]
    return total.reshape(B, 1).astype(np.float32)


# revision 29
# speedup vs baseline: 2.0917x; 1.0354x over previous
"""Trainium2 Bass kernel for nn_Discriminator: MLP + sort-based minibatch
discrimination with gpsimd local_scatter un-permutation. Self-contained."""
import numpy as np
import ml_dtypes

N = 2048
NROWS = 4
NCOLS = 512


def stages(n=None):
    if n is None:
        n = N
    out = []
    p = 1
    while p < n:
        k = p
        while k >= 1:
            lefts = []
            j = k % p
            while j <= n - 1 - k:
                for i in range(0, min(k, n - j - k)):
                    x = i + j
                    if (x // (2 * p)) == ((x + k) // (2 * p)):
                        lefts.append(x)
                j += 2 * k
            out.append((p, k, np.array(sorted(lefts), dtype=np.int64)))
            k //= 2
        p *= 2
    return out


def runs_of(xs):
    """Compress sorted ints into <=3-level pattern (start, L, s1, c1, s2, c2)."""
    xs = np.asarray(xs)
    if len(xs) == 0:
        return None
    breaks = np.where(np.diff(xs) != 1)[0]
    starts_i = np.concatenate([[0], breaks + 1])
    ends_i = np.concatenate([breaks, [len(xs) - 1]])
    run_starts = xs[starts_i]
    run_lens = ends_i - starts_i + 1
    if not np.all(run_lens == run_lens[0]):
        return None
    L = int(run_lens[0])
    if len(run_starts) == 1:
        return (int(run_starts[0]), L, 0, 1, 0, 1)
    d = np.diff(run_starts)
    if np.all(d == d[0]):
        return (int(run_starts[0]), L, int(d[0]), len(run_starts), 0, 1)
    s1 = d[0]
    c1 = 1
    while c1 < len(d) and d[c1 - 1] == s1:
        c1 += 1
    group = c1
    if len(run_starts) % group != 0:
        return None
    rs = run_starts.reshape(-1, group)
    inner = np.diff(rs, axis=1)
    starts2 = rs[:, 0]
    d2 = np.diff(starts2)
    if inner.size and not np.all(inner == s1):
        return None
    if len(d2) and not np.all(d2 == d2[0]):
        return None
    return (int(run_starts[0]), L, int(s1), group,
            int(d2[0]) if len(d2) else 0, len(starts2))


def emit_ops():
    """Returns list of (p, k, [ops]); op = (r0, nrows, drow, colpat, colB0)."""
    all_stages = []
    for (p, k, lefts) in stages():
        ops = []
        rows = lefts // NCOLS
        cols = lefts % NCOLS
        drows = (lefts + k) // NCOLS - rows
        for dr in np.unique(drows):
            sel = drows == dr
            rset = np.unique(rows[sel])
            cset = np.unique(cols[sel])
            assert sel.sum() == len(rset) * len(cset), (p, k, dr)
            for r in rset:
                cc = np.sort(cols[sel & (rows == r)])
                assert np.array_equal(cc, cset), (p, k, dr, r)
            colpat = runs_of(cset)
            assert colpat is not None, (p, k, dr, cset[:20])
            rpat = runs_of(rset)
            assert rpat is not None, (p, k, dr, rset)
            (r0, Lr, sr1, cr1, sr2, cr2) = rpat
            assert sr2 == 0 and cr2 == 1, (p, k, dr, rpat)
            colB0 = int((cset[0] + k) % NCOLS)
            for g in range(cr1):
                rstart = r0 + g * sr1
                ops.append((int(rstart), int(Lr), int(dr), colpat, colB0))
        all_stages.append((p, int(k), ops))
    return all_stages


def _row_chunks(a_base, b_base, nr):
    allowed = {0: 4, 1: 1, 2: 2, 3: 1}
    out = []
    off = 0
    while off < nr:
        c = min(allowed[(a_base + off) % 4], allowed[(b_base + off) % 4], nr - off)
        out.append((off, c))
        off += c
    return out


def legalize(all_stages):
    out = []
    for (p, k, ops) in all_stages:
        nops = []
        for (r0, nr, dr, colpat, colB0) in ops:
            for (off, c) in _row_chunks(r0, r0 + dr, nr):
                nops.append((r0 + off, c, dr, colpat, colB0))
        out.append((p, k, nops))
    return out


def colpat_idx(colpat):
    (c0, L, s1, c1, s2, c2) = colpat
    return (c0 + np.arange(c2)[:, None, None] * s2
            + np.arange(c1)[None, :, None] * s1
            + np.arange(L)[None, None, :]).ravel()


def runs_multi(xs, max_groups=6):
    xs = np.asarray(xs)
    if len(xs) == 0:
        return []
    r = runs_of(xs)
    if r is not None:
        return [r]
    breaks = np.where(np.diff(xs) != 1)[0]
    starts_i = np.concatenate([[0], breaks + 1])
    ends_i = np.concatenate([breaks, [len(xs) - 1]])
    run_starts = xs[starts_i]
    run_lens = ends_i - starts_i + 1
    out = []
    for L in np.unique(run_lens):
        sel = run_lens == L
        rs = run_starts[sel]
        d = np.diff(rs)
        if len(d) == 0 or np.all(d == d[0]):
            out.append((int(rs[0]), int(L), int(d[0]) if len(d) else 0,
                        len(rs), 0, 1))
        else:
            for s in rs:
                out.append((int(s), int(L), 0, 1, 0, 1))
    return out


def emit_pingpong():
    """cp op = (r0, nr, pat, old): old=True -> the cell was untouched in the
    previous stage too, so it can be copied from the 2-stages-old rotation
    buffer (dependency jumps a stage back; copy leaves the critical chain)."""
    out = []
    prev_touched = np.ones((NROWS, NCOLS), dtype=bool)
    for (p, k, ops) in legalize(emit_ops()):
        touched = np.zeros((NROWS, NCOLS), dtype=bool)
        for (r0, nr, dr, colpat, colB0) in ops:
            ia = colpat_idx(colpat)
            ib = ia + (colB0 - colpat[0])
            for rr in range(r0, r0 + nr):
                touched[rr, ia] = True
                touched[rr + dr, ib] = True
        cp_ops = []
        for old in (False, True):
            need = (~touched) & (prev_touched if not old else ~prev_touched)
            r = 0
            while r < NROWS:
                mask = need[r]
                r2 = r + 1
                while r2 < NROWS and np.array_equal(need[r2], mask):
                    r2 += 1
                cols = np.where(mask)[0]
                if len(cols):
                    for pat in runs_multi(cols):
                        off = 0
                        nr_ = r2 - r
                        allowed = {0: 4, 1: 1, 2: 2, 3: 1}
                        while off < nr_:
                            c = min(allowed[(r + off) % 4], nr_ - off)
                            cp_ops.append((r + off, c, pat, old))
                            off += c
                r = r2
        prev_touched = touched
        out.append((p, k, ops, cp_ops))
    return out


def _split_colpat(colpat, max_free=288):
    (c0, L, s1, c1, s2, c2) = colpat
    free = L * c1 * c2
    if free <= max_free:
        return [(0, colpat)]
    if c2 > 1:
        h = c2 // 2
        a = (c0, L, s1, c1, s2, h)
        b = (c0 + h * s2, L, s1, c1, s2, c2 - h)
        return [(d, p) for d0, pp_ in [(0, a), (h * s2, b)]
                for d, p in [(d0 + dd, p2) for dd, p2 in _split_colpat(
                    (pp_[0], pp_[1], pp_[2], pp_[3], pp_[4], pp_[5]), max_free)]]
    if c1 > 1:
        h = c1 // 2
        a = (c0, L, s1, h, 0, 1)
        b = (c0 + h * s1, L, s1, c1 - h, 0, 1)
        out = []
        for base, pat in [(0, a), (h * s1, b)]:
            out.extend(_split_colpat(pat, max_free))
        return out
    h = L // 2
    a = (c0, h, 0, 1, 0, 1)
    b = (c0 + h, L - h, 0, 1, 0, 1)
    return _split_colpat(a, max_free) + _split_colpat(b, max_free)


def drain_split(stages_pp, max_free=288):
    out = []
    for (p, k, cmp_ops, cp_ops) in stages_pp:
        nc_ops = []
        for (r0, nr, dr, colpat, colB0) in cmp_ops:
            for (_, pat) in _split_colpat(colpat, max_free):
                nb0 = colB0 + (pat[0] - colpat[0])
                nc_ops.append((r0, nr, dr, pat, nb0))
        ncp_ops = []
        for (r0, nr, pat, old) in cp_ops:
            for (_, p2) in _split_colpat(pat, max_free):
                ncp_ops.append((r0, nr, p2, old))
        out.append((p, k, nc_ops, ncp_ops))
    return out


def gen_pingpong(n, nrows, ncols, p_min=1, max_free=288):
    global N, NROWS, NCOLS
    oldN, oldR, oldC = N, NROWS, NCOLS
    N, NROWS, NCOLS = n, nrows, ncols
    try:
        full = emit_pingpong()
        filt = [(p, k, c, cp) for (p, k, c, cp) in full if p >= p_min]
        return drain_split(filt, max_free)
    finally:
        N, NROWS, NCOLS = oldN, oldR, oldC


import bass_rust
import concourse.bacc as bacc
import concourse.mybir as mybir
from concourse import tile
from concourse.bass_utils import run_bass_kernel_spmd


B, D, H1, H2, F = 2048, 3072, 512, 256, 100
NCORES = 8
BS = B // NCORES            # 256 rows per core
LEAK = 0.2
P = 128
FL = 13                     # features per core (8*13 = 104 >= 100)
FPAD = NCORES * FL          # 104
NR, NC = 4, 512
RC = 2.0 ** 23              # rounding constant
QLEV = 8190.0
MRANGE = 16.0
QSCALE = QLEV / (2 * MRANGE)
DQ = (2 * MRANGE) / QLEV

f32 = mybir.dt.float32
f16 = mybir.dt.float16
bf16 = mybir.dt.bfloat16
i16 = mybir.dt.int16
i32 = mybir.dt.int32
AF = mybir.ActivationFunctionType
ALU = mybir.AluOpType

KD, K1, K2 = D // P, H1 // P, H2 // P     # 24, 4, 2
NCHUNK = 4                                 # DMA chunks for W1/x
KCH = KD // NCHUNK                         # 6 k-blocks per chunk


def sap(t_ap, pitch, pstart, pcount, coff, colpat):
    """Strided AP view: partitions [pstart, pstart+pcount), free pattern
    colpat=(c0,L,s1,c1,s2,c2) shifted to coff."""
    (c0, L, s1, c1, s2, c2) = colpat
    dims = [(pitch, pcount)]
    if c2 > 1:
        dims.append((s2, c2))
    if c1 > 1:
        dims.append((s1, c1))
    dims.append((1, L))
    a = t_ap.copy()
    a.ap = bass_rust.VecI64Pair(dims)
    a.offset = pstart * pitch + coff
    return a


SRC_OPS = gen_pingpong(256, 1, 256)
MRG_OPS = gen_pingpong(2048, 4, 512, p_min=256, max_free=512)


def emit_sort(nc, ops_table, rowpart, bufs, pitch,
              cp_engines, mir_pool=None):
    """Rotating 3-buffer odd-even merge sort (DVE min/max). The 3-buffer
    rotation avoids WAR stalls between consecutive stages. Cross-row compares
    read the B operand through an SBUF mirror copied by ACT/Pool
    (partition-shifted copies are legal in both directions)."""
    nb = len(bufs)
    ci = 0
    mi = 0
    for si, (p, k, cmp_ops, cp_ops) in enumerate(ops_table):
        cur = bufs[si % nb]
        nxt = bufs[(si + 1) % nb]
        for (r0, nr, pat, old) in cp_ops:
            pa = rowpart * r0
            npart = rowpart * nr
            src = bufs[(si - 1) % nb] if (old and si > 0) else cur
            c_in = sap(src, pitch, pa, npart, pat[0], pat)
            c_out = sap(nxt, pitch, pa, npart, pat[0], pat)
            if old and si > 0:
                cp_engines[ci % len(cp_engines)](c_out, c_in)
                ci += 1
            else:
                # fresh cells: keep the copy on DVE so the stage chain
                # stays on-engine (ACT/Pool copies add ~300ns latency)
                nc.vector.tensor_copy(c_out, c_in)
        for (r0, nr, dr, colpat, colB0) in cmp_ops:
            pa, pb = rowpart * r0, rowpart * (r0 + dr)
            npart = rowpart * nr
            a_in = sap(cur, pitch, pa, npart, colpat[0], colpat)
            a_out = sap(nxt, pitch, pa, npart, colpat[0], colpat)
            b_out = sap(nxt, pitch, pb, npart, colB0, colpat)
            if dr == 0:
                b_in = sap(cur, pitch, pb, npart, colB0, colpat)
            else:
                b_cur = sap(cur, pitch, pb, npart, colB0, colpat)
                mt = mir_pool.tile([128, 512], f32, tag="mir", bufs=4,
                                   name="mirt")
                b_in = sap(mt[:], mt[:].ap[0][0], pa, npart,
                           colpat[0], colpat)
                if mi % 2 == 0:
                    nc.scalar.copy(b_in, b_cur)
                else:
                    nc.gpsimd.tensor_copy(b_in, b_cur)
                mi += 1
            nc.vector.tensor_tensor(a_out, a_in, b_in, ALU.min)
            nc.vector.tensor_tensor(b_out, a_in, b_in, ALU.max)


def build_program():
    nc = bacc.Bacc(
        "TRN2", target_bir_lowering=False, debug=False, num_devices=NCORES)

    SM = 521                       # packed smalls: iota|lmA|lmB|b1|b2|w16|pmask|bf
    WPK = K1 * H2 + K2 * F + K2    # packed W2|T|Wfh

    xTp = nc.dram_tensor("xTp", [P, KD * BS], bf16, kind="ExternalInput").ap()
    W1p = nc.dram_tensor("W1p", [P, KD * H1], bf16, kind="ExternalInput").ap()
    wpkd = nc.dram_tensor("wpk", [P, WPK], bf16, kind="ExternalInput").ap()
    smd = nc.dram_tensor("sm", [P, SM], f32, kind="ExternalInput").ap()
    outc = nc.dram_tensor("outc", [1, B + BS], f32, kind="ExternalOutput").ap()

    with tile.TileContext(nc) as tc:
        with (
            tc.tile_pool(name="persist", bufs=1) as pers,
            tc.tile_pool(name="dram", bufs=1, space="DRAM") as dpool,
        ):
            # ---- packed persistent tile + views ----
            sm_sb = pers.tile([P, SM], f32)
            w16_v = sm_sb[:].bitcast(f16)[:, 2 * 518:2 * 518 + 1]
            pmask_v = sm_sb[:].bitcast(i16)[:, 2 * 519:2 * 519 + 1]
            bq_sb = pers.tile([P, 1], f32)
            nc.vector.memset(bq_sb[:], MRANGE * QSCALE)

            hWf_sb = pers.tile([1, BS], f32)

            # ======== phase 1: MLP (bf16 weights/activations) ========
            with (
                tc.tile_pool(name="mlp", bufs=1) as mp,
                tc.tile_pool(name="psum_mm", bufs=1, space="PSUM") as pmm,
            ):
                wpk_sb = mp.tile([P, WPK], bf16)

                xT_sb = mp.tile([P, KD * BS], bf16)
                W1_sb = mp.tile([P, KD * H1], bf16)
                for c in range(NCHUNK):
                    lo = c * KCH
                    nc.sync.dma_start(
                        W1_sb[:, lo * H1:(lo + KCH) * H1],
                        W1p[:, lo * H1:(lo + KCH) * H1])
                    nc.sync.dma_start(
                        xT_sb[:, lo * BS:(lo + KCH) * BS],
                        xTp[:, lo * BS:(lo + KCH) * BS])
                    if c == 0:
                        nc.sync.dma_start(sm_sb[:], smd)
                        nc.sync.dma_start(wpk_sb[:], wpkd)

                pt1 = [pmm.tile([P, BS], f32, name=f"pt1_{mb}")
                       for mb in range(K1)]
                for k in range(KD):
                    for mb in range(K1):
                        nc.tensor.matmul(
                            pt1[mb][:],
                            W1_sb[:, k * H1 + mb * P: k * H1 + (mb + 1) * P],
                            xT_sb[:, k * BS:(k + 1) * BS],
                            start=(k == 0), stop=(k == KD - 1))
                h1T = [mp.tile([P, BS], bf16, name=f"h1T{m}") for m in range(K1)]
                for mb in range(K1):
                    s1 = mp.tile([P, BS], f32, tag="stmp", bufs=2,
                                 name=f"s1_{mb}")
                    nc.scalar.activation(
                        s1[:], pt1[mb][:], AF.Identity,
                        bias=sm_sb[:, 512 + mb:513 + mb])
                    nc.vector.scalar_tensor_tensor(
                        h1T[mb][:], s1[:], LEAK, s1[:], op0=ALU.mult,
                        op1=ALU.max)

                pt2 = [pmm.tile([P, BS], f32, name=f"pt2_{mb}")
                       for mb in range(K2)]
                for k in range(K1):
                    for mb in range(K2):
                        nc.tensor.matmul(
                            pt2[mb][:],
                            wpk_sb[:, k * H2 + mb * P: k * H2 + (mb + 1) * P],
                            h1T[k][:],
                            start=(k == 0), stop=(k == K1 - 1))
                h2T = [mp.tile([P, BS], bf16, name=f"h2T{m}") for m in range(K2)]
                for mb in range(K2):
                    s2 = mp.tile([P, BS], f32, tag="stmp", bufs=2,
                                 name=f"s2_{mb}")
                    nc.scalar.activation(
                        s2[:], pt2[mb][:], AF.Identity,
                        bias=sm_sb[:, 516 + mb:517 + mb])
                    nc.vector.scalar_tensor_tensor(
                        h2T[mb][:], s2[:], LEAK, s2[:], op0=ALU.mult,
                        op1=ALU.max)

                pt_m = pmm.tile([F, BS], f32, name="ptm")
                for k in range(K2):
                    nc.tensor.matmul(
                        pt_m[:], wpk_sb[:, K1 * H2 + k * F:K1 * H2 + (k + 1) * F],
                        h2T[k][:],
                        start=(k == 0), stop=(k == K2 - 1))

                ph = pmm.tile([1, BS], f32, name="ph")
                for k in range(K2):
                    nc.tensor.matmul(
                        ph[:], wpk_sb[:, K1 * H2 + K2 * F + k:K1 * H2 + K2 * F + k + 1],
                        h2T[k][:],
                        start=(k == 0), stop=(k == K2 - 1))
                nc.vector.tensor_copy(hWf_sb[:], ph[:])

                # ---- quantize + pack straight from PSUM ----
                skey = pers.tile([P, BS], f32)
                nc.scalar.activation(
                    skey[:F, :], pt_m[:], AF.Identity, bias=bq_sb[:F, :],
                    scale=QSCALE)
            sktmp = pers.tile([P, BS], f32)
            sktmp2 = pers.tile([P, BS], f32)
            nc.vector.tensor_scalar(
                skey[:F, :], skey[:F, :], scalar1=RC, scalar2=RC,
                op0=ALU.add, op1=ALU.subtract)
            nc.gpsimd.tensor_scalar(
                skey[:F, :], skey[:F, :], scalar1=8191.0, scalar2=0.0,
                op0=ALU.min, op1=ALU.max)
            nc.vector.tensor_tensor(skey[:F, :], skey[:F, :],
                                    sm_sb[:F, 0:BS], ALU.add)
            spitch = skey[:].ap[0][0]
            emit_sort(nc, SRC_OPS, P, [skey[:], sktmp[:], sktmp2[:]], spitch,
                      [lambda o, i: nc.scalar.copy(o, i),
                       lambda o, i: nc.gpsimd.tensor_copy(o, i)])

            # ======== phase 3: AllToAll ========
            a2a_in = dpool.tile([FPAD, BS], f32)
            a2a_out = dpool.tile([FPAD, BS], f32)
            nc.sync.dma_start(a2a_in[:F, :], skey[:F, :])
            nc.sync.dma_start(a2a_in[F:FPAD, :], skey[:FPAD - F, :])
            nc.gpsimd.collective_compute(
                "AllToAll", ALU.bypass,
                replica_groups=[list(range(NCORES))],
                ins=[a2a_in.opt()], outs=[a2a_out.opt()])

            key = pers.tile([P, NC], f32)
            nc.vector.memset(key[:], 0.0)
            # fancy-AP DMAs (one per quadrant row): a2a_out rows (2r+h)*13+f,
            # col i -> key[32r+f, h*256+i]
            kpitch = key[:].ap[0][0]
            for r in range(4):
                kdst = key[:].copy()
                kdst.ap = bass_rust.VecI64Pair([(kpitch, FL), (1, 2 * BS)])
                kdst.offset = (32 * r) * kpitch
                ksrc = a2a_out[:, :].copy()
                ksrc.ap = bass_rust.VecI64Pair(
                    [(BS, FL), (FL * BS, 2), (1, BS)])
                ksrc.offset = r * 2 * FL * BS
                nc.sync.dma_start(kdst, ksrc)

            # ======== phase 4: merge (30 stages) + scan + unsort ========
            pitch = key[:].ap[0][0]
            with (
                tc.tile_pool(name="sortp", bufs=1) as sp,
                tc.tile_pool(name="psum2", bufs=1, space="PSUM") as pp2,
            ):
                tmp = sp.tile([P, NC], f32)
                tmp2 = sp.tile([P, NC], f32)
                emit_sort(nc, MRG_OPS, 32, [key[:], tmp[:], tmp2[:]], pitch,
                          cp_engines=[lambda o, i: nc.scalar.copy(o, i),
                                      lambda o, i: nc.gpsimd.tensor_copy(o, i)],
                          mir_pool=sp)

                # ---- scan phase: split key = g + j/2048 ----
                kq = sp.tile([P, NC], f32)
                nc.vector.tensor_scalar_mul(kq[:], key[:], 2048.0)
                ki = sp.tile([P, NC], i32)
                nc.vector.tensor_copy(ki[:], kq[:])
                ji = sp.tile([P, NC], i32)
                nc.vector.tensor_scalar(
                    ji[:], ki[:], scalar1=2047, scalar2=None,
                    op0=ALU.bitwise_and)
                ji16 = sp.tile([P, NC], i16)
                nc.gpsimd.tensor_copy(ji16[:], ji[:])
                # ---- unsort index prep (overlaps ACT exps below) ----
                tt = sp.tile([P, NC], i16)
                nc.vector.tensor_scalar(
                    tt[:], ji16[:], scalar1=pmask_v, scalar2=None,
                    op0=ALU.bitwise_or)
                neg1 = sp.tile([P, NC], i16)
                nc.gpsimd.memset(neg1[:], -1)
                m0 = sp.tile([P, NC], i16)
                nc.vector.tensor_scalar(
                    m0[:], tt[:], scalar1=1023, scalar2=None, op0=ALU.is_le)
                idx0 = sp.tile([P, NC], i16)
                nc.vector.select(idx0[:], m0[:], tt[:], neg1[:])
                t1 = sp.tile([P, NC], i16)
                nc.gpsimd.tensor_scalar(
                    t1[:], tt[:], scalar1=1024, scalar2=None, op0=ALU.subtract)
                idx1 = sp.tile([P, NC], i16)
                nc.vector.select(idx1[:], m0[:], neg1[:], t1[:])
                gi = sp.tile([P, NC], i32)
                nc.vector.tensor_scalar(
                    gi[:], ki[:], scalar1=-2048, scalar2=None,
                    op0=ALU.bitwise_and)
                g2k = sp.tile([P, NC], f32)
                nc.vector.tensor_copy(g2k[:], gi[:])
                bneg = sp.tile([P, 1], f32)
                nc.vector.memset(bneg[:], -MRANGE)
                bpos = sp.tile([P, 1], f32)
                nc.vector.memset(bpos[:], MRANGE)
                u = sp.tile([P, NC], f32)
                nc.scalar.activation(
                    u[:], g2k[:], AF.Exp, bias=bneg[:], scale=DQ / 2048.0)
                v = sp.tile([P, NC], f32)
                nc.scalar.activation(
                    v[:], g2k[:], AF.Exp, bias=bpos[:], scale=-DQ / 2048.0)

                su = sp.tile([P, NC], f32)
                nc.vector.tensor_tensor_scan(
                    su[:], u[:], u[:], initial=0.0, op0=ALU.add,
                    op1=ALU.bypass)
                sv = sp.tile([P, NC], f32)
                nc.vector.tensor_tensor_scan(
                    sv[:, NC - 1::-1], v[:, NC - 1::-1], v[:, NC - 1::-1],
                    initial=0.0, op0=ALU.add, op1=ALU.bypass)

                # cross-quadrant carries via masked prefix matmuls
                pcu = pp2.tile([P, 2], f32, name="pcu")
                nc.tensor.matmul(pcu[:, 0:1], sm_sb[:, BS:BS + P], su[:, NC - 1:NC],
                                 start=True, stop=True)
                nc.tensor.matmul(pcu[:, 1:2], sm_sb[:, BS + P:BS + 2 * P], sv[:, 0:1],
                                 start=True, stop=True)
                carr = sp.tile([P, 2], f32)
                nc.vector.tensor_copy(carr[:], pcu[:])

                s1u = sp.tile([P, NC], f32)
                nc.gpsimd.tensor_scalar(
                    s1u[:], su[:], scalar1=carr[:, 0:1], scalar2=None,
                    op0=ALU.add)
                s2vi = sp.tile([P, NC], f32)
                nc.vector.scalar_tensor_tensor(
                    s2vi[:], sv[:], carr[:, 1:2], v[:],
                    op0=ALU.add, op1=ALU.subtract)

                fa = sp.tile([P, NC], f32)
                nc.vector.tensor_tensor(fa[:], v[:], s1u[:], ALU.mult)
                fb = sp.tile([P, NC], f32)
                nc.gpsimd.tensor_tensor(fb[:], u[:], s2vi[:], ALU.mult)
                feats16 = sp.tile([P, NC], f16)
                nc.vector.tensor_tensor(feats16[:], fa[:], fb[:], ALU.add)

                # ---- unsort via local_scatter (j < 1024 | j >= 1024) ----
                dst0 = sp.tile([P, 2 * NC], f16)
                dst1 = sp.tile([P, 2 * NC], f16)
                nc.gpsimd.local_scatter(
                    dst0[:], feats16[:], idx0[:], channels=P,
                    num_elems=2 * NC, num_idxs=NC)
                nc.gpsimd.local_scatter(
                    dst1[:], feats16[:], idx1[:], channels=P,
                    num_elems=2 * NC, num_idxs=NC)

                octile = sp.tile([1, B + BS], f32)
                nc.vector.tensor_scalar(
                    octile[:, B:B + BS], hWf_sb[:],
                    scalar1=sm_sb[0:1, 520:521], scalar2=None, op0=ALU.add)
                for h, dst in ((0, dst0), (1, dst1)):
                    for s in range(2):
                        pc = pp2.tile([1, NC], f32, tag="pc", bufs=2,
                                      name=f"pc{h}{s}")
                        nc.tensor.matmul(
                            pc[:], w16_v, dst[:, s * NC:(s + 1) * NC],
                            start=True, stop=True)
                        oc_sl = octile[:, h * 1024 + s * NC:
                                       h * 1024 + (s + 1) * NC]
                        if s == 0:
                            nc.vector.tensor_copy(oc_sl, pc[:])
                        else:
                            nc.scalar.copy(oc_sl, pc[:])
                nc.sync.dma_start(outc[:], octile[:])

    nc.compile()
    return nc


def _build_in_maps(inputs):
    x = np.asarray(inputs["x"], np.float32)
    W1 = np.asarray(inputs["W1"], np.float32)
    b1 = np.asarray(inputs["b1"], np.float32)
    W2 = np.asarray(inputs["W2"], np.float32)
    b2 = np.asarray(inputs["b2"], np.float32)
    T = np.asarray(inputs["T"], np.float32)
    Wf = np.asarray(inputs["Wf"], np.float32)
    bf = np.asarray(inputs["bf"], np.float32)

    bfl = ml_dtypes.bfloat16
    W1p = np.ascontiguousarray(
        W1.reshape(KD, P, H1).transpose(1, 0, 2).reshape(P, KD * H1)
    ).astype(bfl)
    W2p = W2.reshape(K1, P, H2).transpose(1, 0, 2).reshape(P, K1 * H2)
    Tp = T.reshape(K2, P, F).transpose(1, 0, 2).reshape(P, K2 * F)
    Wfhp = Wf[:H2].reshape(K2, P).T
    wpk = np.ascontiguousarray(
        np.concatenate([W2p, Tp, Wfhp], axis=1)).astype(bfl)
    b1p = b1.reshape(K1, P).T
    b2p = b2.reshape(K2, P).T

    wff = Wf[H2:, 0]
    wff_pad = np.zeros(FPAD, np.float32)
    wff_pad[:F] = wff

    lmaskA = np.zeros((P, P), np.float32)
    lmaskB = np.zeros((P, P), np.float32)
    for k in range(P):
        for m in range(P):
            if k % 32 == m % 32:
                if k // 32 < m // 32:
                    lmaskA[k, m] = 1.0
                elif k // 32 > m // 32:
                    lmaskB[k, m] = 1.0

    pmask = np.full(P, -1, np.int16)
    for r in range(NR):
        pmask[32 * r:32 * r + FL] = 0

    in_maps = []
    for d in range(NCORES):
        m = {"W1p": W1p, "wpk": wpk}
        xT = x[d * BS:(d + 1) * BS, :].T
        m["xTp"] = np.ascontiguousarray(
            xT.reshape(KD, P, BS).transpose(1, 0, 2).reshape(P, KD * BS)
        ).astype(bfl)
        w16 = np.zeros(P, np.float16)
        for r in range(NR):
            w16[32 * r:32 * r + FL] = wff_pad[d * FL:(d + 1) * FL]
        sm = np.zeros((P, 521), np.float32)
        sm[:, 0:BS] = (d * BS + np.arange(BS, dtype=np.float32)) / 2048.0
        sm[:, BS:BS + P] = lmaskA
        sm[:, BS + P:BS + 2 * P] = lmaskB
        sm[:, 512:512 + K1] = b1p
        sm[:, 516:516 + K2] = b2p
        smv = sm.view(np.uint16)
        smv[:, 2 * 518] = w16.view(np.uint16)
        smv[:, 2 * 519] = pmask.view(np.uint16)
        sm[0, 520] = bf[0]
        m["sm"] = sm
        in_maps.append(m)
    return in_maps


_NC_CACHE = None


def _get_program():
    global _NC_CACHE
    if _NC_CACHE is None:
        _NC_CACHE = build_program()
    return _NC_CACHE


def kernel(x, W1, b1, W2, b2, T, Wf, bf):
    nc = _get_program()
    in_maps = _build_in_maps(dict(
        x=x, W1=W1, b1=b1, W2=W2, b2=b2, T=T, Wf=Wf, bf=bf))
    res = run_bass_kernel_spmd(nc, in_maps, core_ids=list(range(NCORES)))
    total = np.zeros(B, np.float64)
    for d in range(NCORES):
        oc = res.results[d]["outc"].ravel()
        total += oc[:B].astype(np.float64)
        total[d * BS:(d + 1) * BS] += oc[B:]
    return total.reshape(B, 1).astype(np.float32)
